# revision 1
# baseline (speedup 1.0000x reference)
"""Causal self-attention with RoPE on 8 Trainium2 NeuronCores.

Sharding: tensor-parallel over heads (4 groups of 4 heads) x data-parallel
over batch (2), one (batch, head-group) pair per core. Each core computes
its heads' QKV projection, RoPE, causal attention, and a row-slice of the
output projection; the host sums the 4 partial projections per batch.

Attention computes scores transposed (k on partitions, q on the free dim,
512-wide q-groups): softmax rowsums come from a ones-vector matmul, the
probabilities feed P@V directly as the moving operand, and no per-block
transposes of the probability matrix are needed.

Matmul operands use float32r (~1.7 cycles/col at N=512 vs ~5 for float32,
measured); accumulation stays fp32 in PSUM. End-to-end relative error vs
the fp32 reference is ~3.5e-4.

Hardcoded problem shape: x (2,2048,2048), Wqkv (2048,6144), Wproj
(2048,2048), cos/sin (2048,64), 16 heads, head_dim 128.
"""

import sys

sys.path.insert(0, "/opt/trn_rl_repo")

import numpy as np

import concourse.bass as bass
import concourse.tile as tile
from concourse import bacc, mybir
from concourse.bass_utils import run_bass_kernel_spmd

B, T, D, H = 2, 2048, 2048, 16
HD, HALF = 128, 64
TPC = 4          # heads per core
NT = T // 128    # 16 t-tiles
NK = D // 128    # 16 contraction chunks for the projections
NG = T // 512    # 4 q-groups per head
SCALE = float(1.0 / np.sqrt(HD))
FP32 = mybir.dt.float32
MM_DT = mybir.dt.float32r
EXP = mybir.ActivationFunctionType.Exp


def build_program():
    nc = bacc.Bacc("TRN2", target_bir_lowering=False, debug=False)

    xT = nc.dram_tensor("xT", [D, T], MM_DT, kind="ExternalInput").ap()
    wqk = nc.dram_tensor("wqk", [D, 2 * TPC * HD], MM_DT, kind="ExternalInput").ap()
    wv = nc.dram_tensor("wv", [D, TPC * HD], MM_DT, kind="ExternalInput").ap()
    wp = nc.dram_tensor("wp", [TPC * HD, D], MM_DT, kind="ExternalInput").ap()
    cos = nc.dram_tensor("cos", [T, HALF], FP32, kind="ExternalInput").ap()
    sin = nc.dram_tensor("sin", [T, HALF], FP32, kind="ExternalInput").ap()
    maskl = nc.dram_tensor("maskl", [128, 128], FP32, kind="ExternalInput").ap()
    ident = nc.dram_tensor("ident", [128, 128], MM_DT, kind="ExternalInput").ap()
    ones = nc.dram_tensor("ones", [128, 1], MM_DT, kind="ExternalInput").ap()
    outT = nc.dram_tensor("outT", [D, T], FP32, kind="ExternalOutput").ap()

    with tile.TileContext(nc) as tc:
        _kernel(tc, xT, wqk, wv, wp, cos, sin, maskl, ident, ones, outT)
    nc.compile()
    return nc


def _kernel(tc, xT, wqk, wv, wp, cos, sin, maskl, ident, ones, outT):
    nc = tc.nc
    NQK = 2 * TPC * HD  # 1024 qk output columns
    NV = TPC * HD       # 512 v output columns

    from contextlib import ExitStack

    with ExitStack() as top:
        # ---- persistent pools ----
        consts = top.enter_context(tc.tile_pool(name="consts", bufs=1))
        qt_pool = top.enter_context(tc.tile_pool(name="qt", bufs=TPC))
        kt_pool = top.enter_context(tc.tile_pool(name="kt", bufs=TPC))
        # PSUM: psS 4 banks + psO 2 + psR 2 = 8
        psS = top.enter_context(tc.tile_pool(name="psS", bufs=3, space="PSUM"))
        psO = top.enter_context(tc.tile_pool(name="psO", bufs=3, space="PSUM"))
        psR = top.enter_context(tc.tile_pool(name="psR", bufs=2, space="PSUM"))

        l_tile = consts.tile([128, 128], FP32)
        nc.sync.dma_start(out=l_tile, in_=maskl)
        id_tile = consts.tile([128, 128], MM_DT)
        nc.sync.dma_start(out=id_tile, in_=ident)
        ones_t = consts.tile([128, 1], MM_DT)
        nc.sync.dma_start(out=ones_t, in_=ones)

        # QT/KT: per head, (128 hd, T)
        QT = [qt_pool.tile([128, T], MM_DT, tag="qt", name=f"QT{i}") for i in range(TPC)]
        KT = [kt_pool.tile([128, T], MM_DT, tag="kt", name=f"KT{i}") for i in range(TPC)]

        # ================= phase 1a: qk projection + rope + transpose ======
        with tc.tile_pool(name="wqk_cache", bufs=NK) as wqk_pool, \
             tc.tile_pool(name="x_stream", bufs=32) as x_pool, \
             tc.tile_pool(name="qk_evict", bufs=4) as qk_pool, \
             tc.tile_pool(name="rope_tmp", bufs=8) as rope_pool, \
             tc.tile_pool(name="cs", bufs=4) as cs_pool:

            # cache all of wqk in SBUF (8MB), reused by all 16 t-tiles
            WQK = []
            for k in range(NK):
                w = wqk_pool.tile([128, NQK], MM_DT, tag="wqk")
                nc.sync.dma_start(out=w, in_=wqk[k * 128 : (k + 1) * 128, :])
                WQK.append(w)

            for t in range(NT):
                psQ = psS.tile([128, 512], FP32, tag="psS")
                psK = psS.tile([128, 512], FP32, tag="psS")
                for k in range(NK):
                    xt = x_pool.tile([128, 128], MM_DT, tag="x")
                    nc.sync.dma_start(
                        out=xt, in_=xT[k * 128 : (k + 1) * 128, t * 128 : (t + 1) * 128]
                    )
                    nc.tensor.matmul(psQ, xt, WQK[k][:, 0:512],
                                     start=(k == 0), stop=(k == NK - 1))
                    nc.tensor.matmul(psK, xt, WQK[k][:, 512:1024],
                                     start=(k == 0), stop=(k == NK - 1))
                ct = cs_pool.tile([128, HALF], FP32, tag="c")
                nc.sync.dma_start(out=ct, in_=cos[t * 128 : (t + 1) * 128, :])
                st = cs_pool.tile([128, HALF], FP32, tag="s")
                nc.sync.dma_start(out=st, in_=sin[t * 128 : (t + 1) * 128, :])
                # broadcast (128, 64) -> (128, 2, 64) with 0-step middle dim
                c_b = ct.unsqueeze(1).broadcast_to((128, 2, HALF))
                s_b = st.unsqueeze(1).broadcast_to((128, 2, HALF))

                for hh in range(TPC):
                    for which in range(2):  # 0 = Q, 1 = K
                        # rope reads the projection psum directly (DVE can
                        # read PSUM); no SBUF eviction hop needed
                        srcp = psQ if which == 0 else psK
                        blk = srcp[:, hh * HD : (hh + 1) * HD]
                        pair = blk.rearrange("p (two h) -> p two h", two=2)
                        t_a = rope_pool.tile([128, 2, HALF], FP32, tag="ta")
                        t_b = rope_pool.tile([128, 2, HALF], FP32, tag="tb")
                        nc.vector.tensor_mul(t_a, pair, c_b)
                        nc.vector.tensor_mul(t_b, pair, s_b)
                        ro = rope_pool.tile([128, HALF, 2], MM_DT, tag="ro")
                        # out[:, i, 0] = x1*c - x2*s ; out[:, i, 1] = x1*s + x2*c
                        nc.vector.tensor_sub(ro[:, :, 0:1],
                                             t_a[:, 0, :].unsqueeze(2),
                                             t_b[:, 1, :].unsqueeze(2))
                        nc.vector.tensor_add(ro[:, :, 1:2],
                                             t_b[:, 0, :].unsqueeze(2),
                                             t_a[:, 1, :].unsqueeze(2))
                        ro_flat = ro.rearrange("p h two -> p (h two)")
                        # transpose (128t, 128hd) -> (128hd, 128t)
                        pst = psO.tile([128, 512], MM_DT, tag="psO")
                        nc.tensor.transpose(pst[:, :128], ro_flat, id_tile)
                        dst = QT[hh] if which == 0 else KT[hh]
                        # evict on ACT: DVE is the loaded engine in this phase
                        nc.scalar.copy(
                            out=dst[:, t * 128 : (t + 1) * 128], in_=pst[:, :128]
                        )

        # ========== phase 2: v projection interleaved with attention ======
        # V: per t-tile, (128 t, 512) with 4 head column groups. The V
        # projection for q-group g's new t-tiles is emitted just before
        # group g's attention so its matmuls fill PE slack left by the
        # exp-paced attention pipeline (and its xT DMAs spread out).
        v_pool = top.enter_context(tc.tile_pool(name="v", bufs=NT))
        V = [v_pool.tile([128, NV], MM_DT, tag="v", name=f"V{i}") for i in range(NT)]
        # attention output transposed: per head, (128 hd, T)
        o_pool = top.enter_context(tc.tile_pool(name="o", bufs=TPC))
        OT = [o_pool.tile([128, T], MM_DT, tag="o", name=f"OT{i}") for i in range(TPC)]
        with tc.tile_pool(name="wv_cache", bufs=NK) as wv_pool, \
             tc.tile_pool(name="x_stream2", bufs=20) as x2_pool, \
             tc.tile_pool(name="p_sb", bufs=6) as p_pool, \
             tc.tile_pool(name="rs_sb", bufs=4) as rs_pool, \
             tc.tile_pool(name="rb_sb", bufs=3) as rb_pool:
            WV = []
            for k in range(NK):
                w = wv_pool.tile([128, NV], MM_DT, tag="wv")
                nc.sync.dma_start(out=w, in_=wv[k * 128 : (k + 1) * 128, :])
                WV.append(w)
            for g in range(NG):
                for t in range(4 * g, 4 * g + 4):
                    ps = psS.tile([128, 512], FP32, tag="psS")
                    for k in range(NK):
                        xt = x2_pool.tile([128, 128], MM_DT, tag="x2")
                        nc.sync.dma_start(
                            out=xt,
                            in_=xT[k * 128 : (k + 1) * 128, t * 128 : (t + 1) * 128],
                        )
                        nc.tensor.matmul(ps, xt, WV[k],
                                         start=(k == 0), stop=(k == NK - 1))
                    nc.vector.tensor_copy(out=V[t], in_=ps)
                for hh in range(TPC):
                    qcol0 = g * 512
                    nchunks = 4 * g + 4
                    po = psO.tile([128, 512], FP32, tag="psO")
                    rs = psR.tile([1, 512], FP32, tag="psR")
                    for kj in range(nchunks):
                        s0 = max(0, kj - 4 * g)   # first unmasked 128-q sub
                        off = s0 * 128
                        w = 512 - off
                        ps = psS.tile([128, 512], FP32, tag="psS")
                        # scoresT chunk (128 k, w q)
                        nc.tensor.matmul(
                            ps[:, :w],
                            KT[hh][:, kj * 128 : (kj + 1) * 128],
                            QT[hh][:, qcol0 + off : qcol0 + 512],
                            start=True, stop=True,
                        )
                        sd = kj - 4 * g
                        if 0 <= sd <= 3:
                            dcol = sd * 128 - off
                            nc.vector.tensor_sub(
                                ps[:, dcol : dcol + 128],
                                ps[:, dcol : dcol + 128],
                                l_tile,
                            )
                        pt = p_pool.tile([128, 512], MM_DT, tag="p")
                        nc.scalar.activation(out=pt[:, :w], in_=ps[:, :w],
                                             func=EXP, scale=SCALE)
                        # rowsum over k (ones-vector matmul), psum-accumulated
                        nc.tensor.matmul(rs[:, off:512], ones_t, pt[:, :w],
                                         start=(kj == 0), stop=(kj == nchunks - 1))
                        # PV accumulate: (128 hd, w q)
                        nc.tensor.matmul(po[:, off:512],
                                         V[kj][:, hh * HD : (hh + 1) * HD],
                                         pt[:, :w],
                                         start=(kj == 0), stop=(kj == nchunks - 1))
                    rr = rs_pool.tile([1, 512], FP32, tag="rr")
                    nc.vector.tensor_copy(out=rr, in_=rs)
                    rrep = rb_pool.tile([128, 512], FP32, tag="rrep")
                    nc.gpsimd.partition_broadcast(rrep, rr)
                    nc.vector.reciprocal(rrep, rrep)
                    nc.vector.tensor_mul(OT[hh][:, qcol0 : qcol0 + 512], po, rrep)

        # ================= phase 3: output projection =====================
        with tc.tile_pool(name="wp_stream", bufs=8) as wp_pool, \
             tc.tile_pool(name="out_evict", bufs=4) as out_pool:
            for m in range(NK):  # 16 blocks of 128 output (D) rows
                WPm = []
                for hh in range(TPC):
                    wt = wp_pool.tile([128, 128], MM_DT, tag="wp", name=f"wt{m}_{hh}")
                    nc.sync.dma_start(
                        out=wt,
                        in_=wp[hh * 128 : (hh + 1) * 128, m * 128 : (m + 1) * 128],
                    )
                    WPm.append(wt)
                for c4 in range(4):  # 512-wide T chunks
                    ps = psO.tile([128, 512], FP32, tag="psO")
                    for hh in range(TPC):
                        nc.tensor.matmul(
                            ps,
                            WPm[hh],
                            OT[hh][:, c4 * 512 : (c4 + 1) * 512],
                            start=(hh == 0), stop=(hh == TPC - 1),
                        )
                    ob = out_pool.tile([128, 512], FP32, tag="ob")
                    nc.scalar.copy(out=ob, in_=ps)
                    nc.sync.dma_start(
                        out=outT[m * 128 : (m + 1) * 128, c4 * 512 : (c4 + 1) * 512],
                        in_=ob,
                    )




_PROGRAM = None


def _get_program():
    global _PROGRAM
    if _PROGRAM is None:
        _PROGRAM = build_program()
    return _PROGRAM


def _make_in_maps(x, cos, sin, Wqkv, Wproj):
    maskl = (np.tril(np.ones((128, 128), np.float32), -1) * 1e30).astype(np.float32)
    ident = np.eye(128, dtype=np.float32)
    ones = np.ones((128, 1), dtype=np.float32)
    in_maps = []
    for c in range(8):
        b, hg = c // 4, c % 4
        h0 = hg * TPC
        in_maps.append({
            "xT": np.ascontiguousarray(x[b].T),
            "wqk": np.ascontiguousarray(np.concatenate(
                [Wqkv[:, h0 * HD : (h0 + TPC) * HD],
                 Wqkv[:, D + h0 * HD : D + (h0 + TPC) * HD]], axis=1)),
            "wv": np.ascontiguousarray(Wqkv[:, 2 * D + h0 * HD : 2 * D + (h0 + TPC) * HD]),
            "wp": np.ascontiguousarray(Wproj[h0 * HD : (h0 + TPC) * HD, :]),
            "cos": np.asarray(cos, np.float32),
            "sin": np.asarray(sin, np.float32),
            "maskl": maskl,
            "ident": ident,
            "ones": ones,
        })
    return in_maps


def _combine(results):
    outs = []
    for b in range(2):
        acc = results[4 * b]["outT"].astype(np.float32)
        for hg in range(1, 4):
            acc = acc + results[4 * b + hg]["outT"]
        outs.append(acc.T)
    return np.ascontiguousarray(np.stack(outs))


def kernel(x, cos, sin, Wqkv, Wproj):
    nc = _get_program()
    in_maps = _make_in_maps(np.asarray(x, np.float32), cos, sin,
                            np.asarray(Wqkv, np.float32), np.asarray(Wproj, np.float32))
    res = run_bass_kernel_spmd(nc, in_maps, list(range(8)))
    return _combine(res.results)


def _install_ntff_shim():
    """Provide the antenv.axon_hooks registry this container lacks, wired to
    the ctypes NTFF hook from trn_agent_boot, so trace=True works."""
    import types

    if "antenv.axon_hooks" in sys.modules:
        return
    hook = None
    try:
        from trn_agent_boot.trn_boot import _ntff_profile_via_ctypes
        hook = _ntff_profile_via_ctypes("/opt/axon/libaxon_pjrt.so")
    except Exception as e:
        print("ntff shim unavailable:", e)
    mod = types.ModuleType("antenv.axon_hooks")
    mod._hook = hook
    mod.get_axon_ntff_profile_hook = lambda: mod._hook
    mod.set_axon_ntff_profile_hook = lambda h: setattr(mod, "_hook", h)
    sys.modules["antenv.axon_hooks"] = mod
    # keep artifacts local; the bucket upload path isn't available here
    import concourse.bass_utils as bu
    bu.upload_artifacts = lambda tmpdir: tmpdir


def kernel_profiled(x, cos, sin, Wqkv, Wproj, trace_cores=None, tmpdir=None):
    nc = _get_program()
    _install_ntff_shim()
    in_maps = _make_in_maps(np.asarray(x, np.float32), cos, sin,
                            np.asarray(Wqkv, np.float32), np.asarray(Wproj, np.float32))
    res = run_bass_kernel_spmd(nc, in_maps, list(range(8)), trace=True,
                               trace_cores=trace_cores, tmpdir=tmpdir)
    return _combine(res.results), res



# revision 4
# speedup vs baseline: 1.4190x; 1.4190x over previous
"""Causal self-attention with RoPE on 8 Trainium2 NeuronCores.

Sharding: tensor-parallel over heads (4 groups of 4 heads) x data-parallel
over batch (2), one (batch, head-group) pair per core. Each core computes
its heads' QKV projection, RoPE, causal attention, and a row-slice of the
output projection; the host sums the 4 partial projections per batch.

All matmul operands are bf16 (fp32 PSUM accumulation). Q^T/K^T are
computed directly with the weight chunk stationary and x^T moving, so no
PE transposes are needed; RoPE is applied in the transposed (head-dim on
partitions) layout, exploiting that any fixed permutation of the head dim
cancels in q.k (rotate-half instead of interleave). The whole kernel is a
single fused pass over four 512-token groups: QKV projection -> RoPE ->
causal attention for that q-group -> output-projection column chunk, so
projection, attention, exp, and DMA all overlap.

Hardcoded problem shape: x (2,2048,2048), Wqkv (2048,6144), Wproj
(2048,2048), cos/sin (2048,64), 16 heads, head_dim 128.
"""

import sys

sys.path.insert(0, "/opt/trn_rl_repo")

import ml_dtypes
import numpy as np

import concourse.bass as bass
import concourse.tile as tile
from concourse import bacc, mybir
from concourse.bass_utils import run_bass_kernel_spmd

B, T, D, H = 2, 2048, 2048, 16
HD, HALF = 128, 64
TPC = 4          # heads per core
NK = D // 128    # 16 contraction chunks for the projections
NG = T // 512    # 4 q/t-groups
NT = T // 128    # 16 key tiles
SCALE = float(1.0 / np.sqrt(HD))
FP32 = mybir.dt.float32
BF16 = mybir.dt.bfloat16
BF16_NP = ml_dtypes.bfloat16
EXP = mybir.ActivationFunctionType.Exp


def build_program():
    nc = bacc.Bacc("TRN2", target_bir_lowering=False, debug=False)

    xT = nc.dram_tensor("xT", [D, T], BF16, kind="ExternalInput").ap()
    wq = nc.dram_tensor("wq", [D, TPC * HD], BF16, kind="ExternalInput").ap()
    wk = nc.dram_tensor("wk", [D, TPC * HD], BF16, kind="ExternalInput").ap()
    wv = nc.dram_tensor("wv", [D, TPC * HD], BF16, kind="ExternalInput").ap()
    wp = nc.dram_tensor("wp", [TPC * HD, D], BF16, kind="ExternalInput").ap()
    cosT = nc.dram_tensor("cosT", [HALF, T], FP32, kind="ExternalInput").ap()
    sinT = nc.dram_tensor("sinT", [HALF, T], FP32, kind="ExternalInput").ap()
    maskl = nc.dram_tensor("maskl", [128, 128], FP32, kind="ExternalInput").ap()
    ones = nc.dram_tensor("ones", [128, 1], BF16, kind="ExternalInput").ap()
    outT = nc.dram_tensor("outT", [D, T], FP32, kind="ExternalOutput").ap()

    with tile.TileContext(nc) as tc:
        _kernel(tc, xT, wq, wk, wv, wp, cosT, sinT, maskl, ones, outT)
    nc.compile()
    return nc


def _kernel(tc, xT, wq, wk, wv, wp, cosT, sinT, maskl, ones, outT):
    nc = tc.nc
    from contextlib import ExitStack

    with ExitStack() as top:
        consts = top.enter_context(tc.tile_pool(name="consts", bufs=1))
        wq_pool = top.enter_context(tc.tile_pool(name="wq", bufs=NK))
        wk_pool = top.enter_context(tc.tile_pool(name="wk", bufs=NK))
        wv_pool = top.enter_context(tc.tile_pool(name="wv", bufs=NK))
        wp_pool = top.enter_context(tc.tile_pool(name="wp", bufs=16))
        x_pool = top.enter_context(tc.tile_pool(name="x", bufs=20))
        qt_pool = top.enter_context(tc.tile_pool(name="qt", bufs=TPC))
        kt_pool = top.enter_context(tc.tile_pool(name="kt", bufs=TPC))
        v_pool = top.enter_context(tc.tile_pool(name="v", bufs=NT))
        o_pool = top.enter_context(tc.tile_pool(name="o", bufs=TPC))
        rope_pool = top.enter_context(tc.tile_pool(name="rope", bufs=2))
        p_pool = top.enter_context(tc.tile_pool(name="p", bufs=4))
        rs_pool = top.enter_context(tc.tile_pool(name="rs", bufs=2))
        rb_pool = top.enter_context(tc.tile_pool(name="rb", bufs=2))
        ob_pool = top.enter_context(tc.tile_pool(name="ob", bufs=3))
        # PSUM: pp 4 + po 2 + pr 2 = 8 banks
        pp = top.enter_context(tc.tile_pool(name="pp", bufs=4, space="PSUM"))
        ppo = top.enter_context(tc.tile_pool(name="ppo", bufs=2, space="PSUM"))
        ppr = top.enter_context(tc.tile_pool(name="ppr", bufs=2, space="PSUM"))

        l_tile = consts.tile([128, 128], FP32)
        nc.sync.dma_start(out=l_tile, in_=maskl)
        ones_t = consts.tile([128, 1], BF16)
        nc.sync.dma_start(out=ones_t, in_=ones)
        cosT_t = consts.tile([HALF, T], FP32)
        nc.sync.dma_start(out=cosT_t, in_=cosT)
        sinT_t = consts.tile([HALF, T], FP32)
        nc.sync.dma_start(out=sinT_t, in_=sinT)

        # DMA order matters for warmup: wq, then group-0 x, then wk/wv/wp.
        WQ = []
        for k in range(NK):
            w = wq_pool.tile([128, TPC * HD], BF16, tag="wq")
            nc.sync.dma_start(out=w, in_=wq[k * 128 : (k + 1) * 128, :])
            WQ.append(w)
        X0 = []
        for k in range(NK):
            xt = x_pool.tile([128, 512], BF16, tag="x")
            nc.sync.dma_start(out=xt, in_=xT[k * 128 : (k + 1) * 128, 0:512])
            X0.append(xt)
        WK = []
        for k in range(NK):
            w = wk_pool.tile([128, TPC * HD], BF16, tag="wk")
            nc.sync.dma_start(out=w, in_=wk[k * 128 : (k + 1) * 128, :])
            WK.append(w)
        WV = []
        for k in range(NK):
            w = wv_pool.tile([128, TPC * HD], BF16, tag="wv")
            nc.sync.dma_start(out=w, in_=wv[k * 128 : (k + 1) * 128, :])
            WV.append(w)
        WP = []  # index hh*4 + m4 -> wp[hh*128:(hh+1)*128, m4*512:(m4+1)*512]
        for hh in range(TPC):
            for m4 in range(4):
                w = wp_pool.tile([128, 512], BF16, tag="wp")
                nc.sync.dma_start(
                    out=w,
                    in_=wp[hh * 128 : (hh + 1) * 128, m4 * 512 : (m4 + 1) * 512],
                )
                WP.append(w)

        QT = [qt_pool.tile([128, T], BF16, tag="qt", name=f"QT{i}") for i in range(TPC)]
        KT = [kt_pool.tile([128, T], BF16, tag="kt", name=f"KT{i}") for i in range(TPC)]
        V = [v_pool.tile([128, TPC * HD], BF16, tag="v", name=f"V{i}") for i in range(NT)]
        OT = [o_pool.tile([128, T], BF16, tag="o", name=f"OT{i}") for i in range(TPC)]

        for g in range(NG):
            c0, c1 = g * 512, (g + 1) * 512
            if g == 0:
                XG = X0
            else:
                XG = []
                for k in range(NK):
                    xt = x_pool.tile([128, 512], BF16, tag="x")
                    nc.sync.dma_start(
                        out=xt, in_=xT[k * 128 : (k + 1) * 128, c0:c1]
                    )
                    XG.append(xt)
            cT = cosT_t[:, c0:c1]
            sT = sinT_t[:, c0:c1]

            # ---- Q/K projection (transposed output) + rope ----
            for hh in range(TPC):
                for Wsrc, dstT in ((WQ, QT), (WK, KT)):
                    ps = pp.tile([128, 512], FP32, tag="pp")
                    for k in range(NK):
                        nc.tensor.matmul(
                            ps,
                            Wsrc[k][:, hh * 128 : (hh + 1) * 128],
                            XG[k],
                            start=(k == 0),
                            stop=(k == NK - 1),
                        )
                    # rope: out_lo = q1*c - q2*s ; out_hi = q1*s + q2*c
                    q1 = ps[0:HALF, :]
                    q2 = ps[HALF:128, :]
                    t1 = rope_pool.tile([HALF, 512], FP32, tag="t1")
                    t2 = rope_pool.tile([HALF, 512], FP32, tag="t2")
                    t3 = rope_pool.tile([HALF, 512], FP32, tag="t3")
                    t4 = rope_pool.tile([HALF, 512], FP32, tag="t4")
                    nc.vector.tensor_mul(t1, q1, cT)
                    nc.vector.tensor_mul(t2, q2, sT)
                    nc.vector.tensor_mul(t3, q1, sT)
                    nc.vector.tensor_mul(t4, q2, cT)
                    nc.vector.tensor_sub(dstT[hh][0:HALF, c0:c1], t1, t2)
                    nc.vector.tensor_add(dstT[hh][HALF:128, c0:c1], t3, t4)

            # ---- V projection ----
            for tt in range(4):
                ps = pp.tile([128, 512], FP32, tag="pp")
                for k in range(NK):
                    nc.tensor.matmul(
                        ps,
                        XG[k][:, tt * 128 : (tt + 1) * 128],
                        WV[k],
                        start=(k == 0),
                        stop=(k == NK - 1),
                    )
                nc.scalar.copy(out=V[4 * g + tt], in_=ps)

            # ---- causal attention for q-group g ----
            for hh in range(TPC):
                nchunks = 4 * g + 4
                po = ppo.tile([128, 512], FP32, tag="po")
                rs = ppr.tile([1, 512], FP32, tag="rs")
                for kj in range(nchunks):
                    s0 = max(0, kj - 4 * g)   # first unmasked 128-q sub
                    off = s0 * 128
                    w = 512 - off
                    ps = pp.tile([128, 512], FP32, tag="pp")
                    # scoresT chunk (128 k, w q)
                    nc.tensor.matmul(
                        ps[:, :w],
                        KT[hh][:, kj * 128 : (kj + 1) * 128],
                        QT[hh][:, c0 + off : c1],
                        start=True,
                        stop=True,
                    )
                    sd = kj - 4 * g
                    if 0 <= sd <= 3:
                        dcol = sd * 128 - off
                        nc.vector.tensor_sub(
                            ps[:, dcol : dcol + 128],
                            ps[:, dcol : dcol + 128],
                            l_tile,
                        )
                    pt = p_pool.tile([128, 512], BF16, tag="p")
                    nc.scalar.activation(out=pt[:, :w], in_=ps[:, :w],
                                         func=EXP, scale=SCALE)
                    # rowsum over k (ones-vector matmul), psum-accumulated
                    nc.tensor.matmul(rs[:, off:512], ones_t, pt[:, :w],
                                     start=(kj == 0), stop=(kj == nchunks - 1))
                    # PV accumulate: (128 hd, w q)
                    nc.tensor.matmul(po[:, off:512],
                                     V[kj][:, hh * HD : (hh + 1) * HD],
                                     pt[:, :w],
                                     start=(kj == 0), stop=(kj == nchunks - 1))
                rr = rs_pool.tile([1, 512], FP32, tag="rr")
                nc.vector.tensor_copy(out=rr, in_=rs)
                nc.vector.reciprocal(rr, rr)
                rrep = rb_pool.tile([128, 512], FP32, tag="rb")
                nc.gpsimd.partition_broadcast(rrep, rr)
                nc.vector.tensor_mul(OT[hh][:, c0:c1], po, rrep)

            # ---- output projection for this 512-token chunk ----
            for m in range(NK):
                ps = pp.tile([128, 512], FP32, tag="pp")
                for hh in range(TPC):
                    nc.tensor.matmul(
                        ps,
                        WP[hh * 4 + m // 4][:, (m % 4) * 128 : (m % 4 + 1) * 128],
                        OT[hh][:, c0:c1],
                        start=(hh == 0),
                        stop=(hh == TPC - 1),
                    )
                ob = ob_pool.tile([128, 512], FP32, tag="ob")
                nc.vector.tensor_copy(out=ob, in_=ps)
                nc.sync.dma_start(
                    out=outT[m * 128 : (m + 1) * 128, c0:c1], in_=ob
                )


_PROGRAM = None


def _get_program():
    global _PROGRAM
    if _PROGRAM is None:
        _PROGRAM = build_program()
    return _PROGRAM


def _make_in_maps(x, cos, sin, Wqkv, Wproj):
    maskl = (np.tril(np.ones((128, 128), np.float32), -1) * 1e30).astype(np.float32)
    ones = np.ones((128, 1), dtype=BF16_NP)
    cosT = np.ascontiguousarray(np.asarray(cos, np.float32).T)
    sinT = np.ascontiguousarray(np.asarray(sin, np.float32).T)
    in_maps = []
    for c in range(8):
        b, hg = c // 4, c % 4
        h0 = hg * TPC
        in_maps.append({
            "xT": np.ascontiguousarray(x[b].T.astype(BF16_NP)),
            "wq": np.ascontiguousarray(
                Wqkv[:, h0 * HD : (h0 + TPC) * HD].astype(BF16_NP)),
            "wk": np.ascontiguousarray(
                Wqkv[:, D + h0 * HD : D + (h0 + TPC) * HD].astype(BF16_NP)),
            "wv": np.ascontiguousarray(
                Wqkv[:, 2 * D + h0 * HD : 2 * D + (h0 + TPC) * HD].astype(BF16_NP)),
            "wp": np.ascontiguousarray(
                Wproj[h0 * HD : (h0 + TPC) * HD, :].astype(BF16_NP)),
            "cosT": cosT,
            "sinT": sinT,
            "maskl": maskl,
            "ones": ones,
        })
    return in_maps


def _combine(results):
    outs = []
    for b in range(2):
        acc = results[4 * b]["outT"].astype(np.float32)
        for hg in range(1, 4):
            acc = acc + results[4 * b + hg]["outT"]
        outs.append(acc.T)
    return np.ascontiguousarray(np.stack(outs))


def kernel(x, cos, sin, Wqkv, Wproj):
    nc = _get_program()
    in_maps = _make_in_maps(np.asarray(x, np.float32), cos, sin,
                            np.asarray(Wqkv, np.float32), np.asarray(Wproj, np.float32))
    res = run_bass_kernel_spmd(nc, in_maps, list(range(8)))
    return _combine(res.results)


def _install_ntff_shim():
    """Provide the antenv.axon_hooks registry this container lacks, wired to
    the ctypes NTFF hook from trn_agent_boot, so trace=True works."""
    import types

    if "antenv.axon_hooks" in sys.modules:
        return
    hook = None
    try:
        from trn_agent_boot.trn_boot import _ntff_profile_via_ctypes
        hook = _ntff_profile_via_ctypes("/opt/axon/libaxon_pjrt.so")
    except Exception as e:
        print("ntff shim unavailable:", e)
    mod = types.ModuleType("antenv.axon_hooks")
    mod._hook = hook
    mod.get_axon_ntff_profile_hook = lambda: mod._hook
    mod.set_axon_ntff_profile_hook = lambda h: setattr(mod, "_hook", h)
    sys.modules["antenv.axon_hooks"] = mod
    # keep artifacts local; the bucket upload path isn't available here
    import concourse.bass_utils as bu
    bu.upload_artifacts = lambda tmpdir: tmpdir


def kernel_profiled(x, cos, sin, Wqkv, Wproj, trace_cores=None, tmpdir=None):
    nc = _get_program()
    _install_ntff_shim()
    in_maps = _make_in_maps(np.asarray(x, np.float32), cos, sin,
                            np.asarray(Wqkv, np.float32), np.asarray(Wproj, np.float32))
    res = run_bass_kernel_spmd(nc, in_maps, list(range(8)), trace=True,
                               trace_cores=trace_cores, tmpdir=tmpdir)
    return _combine(res.results), res


# revision 12
# speedup vs baseline: 1.4365x; 1.0124x over previous
"""Causal self-attention with RoPE on 8 Trainium2 NeuronCores.

Sharding: tensor-parallel over heads (4 groups of 4 heads) x data-parallel
over batch (2), one (batch, head-group) pair per core. Each core computes
its heads' QKV projection, RoPE, causal attention, and a row-slice of the
output projection; the host sums the 4 partial projections per batch.

All matmul operands are bf16 (fp32 PSUM accumulation). Q^T/K^T are
computed directly with the weight chunk stationary and x^T moving, so no
PE transposes are needed; RoPE is applied in the transposed (head-dim on
partitions) layout, exploiting that any fixed permutation of the head dim
cancels in q.k (rotate-half instead of interleave). RoPE uses packed
cos/sin tables ([cos;sin] and [sin;cos] on partition halves) so it's 2
full-width DVE muls + 2 half-width add/sub.

Schedule is software-pipelined on the tensor queue: the projection block
for group g+1 is issued before attention of group g, and inside attention
the scores+exp pass of head h+1 is interleaved chunk-by-chunk with the
rowsum+PV pass of head h, so the PE never sits at the queue head waiting
for an exp. x/table DMAs go through the scalar-engine DGE queue, weights
through the sync queue, halving the warmup.

Hardcoded problem shape: x (2,2048,2048), Wqkv (2048,6144), Wproj
(2048,2048), cos/sin (2048,64), 16 heads, head_dim 128.
"""

import sys

sys.path.insert(0, "/opt/trn_rl_repo")

import ml_dtypes
import numpy as np

import concourse.bass as bass
import concourse.tile as tile
from concourse import bacc, mybir
from concourse.bass_utils import run_bass_kernel_spmd

B, T, D, H = 2, 2048, 2048, 16
HD, HALF = 128, 64
TPC = 4          # heads per core
NK = D // 128    # 16 contraction chunks for the projections
NG = T // 512    # 4 q/t-groups
NT = T // 128    # 16 key tiles
SCALE = float(1.0 / np.sqrt(HD))
FP32 = mybir.dt.float32
BF16 = mybir.dt.bfloat16
BF16_NP = ml_dtypes.bfloat16
EXP = mybir.ActivationFunctionType.Exp


def build_program():
    nc = bacc.Bacc("TRN2", target_bir_lowering=False, debug=False)

    xT = nc.dram_tensor("xT", [D, T], BF16, kind="ExternalInput").ap()
    wq = nc.dram_tensor("wq", [D, TPC * HD], BF16, kind="ExternalInput").ap()
    wk = nc.dram_tensor("wk", [D, TPC * HD], BF16, kind="ExternalInput").ap()
    wv = nc.dram_tensor("wv", [D, TPC * HD], BF16, kind="ExternalInput").ap()
    wp = nc.dram_tensor("wp", [TPC * HD, D], BF16, kind="ExternalInput").ap()
    cs = nc.dram_tensor("cs", [128, T], FP32, kind="ExternalInput").ap()
    maskl = nc.dram_tensor("maskl", [128, 128], FP32, kind="ExternalInput").ap()
    ones = nc.dram_tensor("ones", [128, 1], BF16, kind="ExternalInput").ap()
    outT = nc.dram_tensor("outT", [D, T], FP32, kind="ExternalOutput").ap()

    with tile.TileContext(nc) as tc:
        _kernel(tc, xT, wq, wk, wv, wp, cs, maskl, ones, outT)
    nc.compile()
    return nc


def _kernel(tc, xT, wq, wk, wv, wp, cs, maskl, ones, outT):
    nc = tc.nc
    from contextlib import ExitStack

    with ExitStack() as top:
        consts = top.enter_context(tc.tile_pool(name="consts", bufs=1))
        wq_pool = top.enter_context(tc.tile_pool(name="wq", bufs=NK))
        wk_pool = top.enter_context(tc.tile_pool(name="wk", bufs=NK))
        wv_pool = top.enter_context(tc.tile_pool(name="wv", bufs=NK))
        wp_pool = top.enter_context(tc.tile_pool(name="wp", bufs=16))
        x_pool = top.enter_context(tc.tile_pool(name="x", bufs=17))
        qt_pool = top.enter_context(tc.tile_pool(name="qt", bufs=TPC))
        kt_pool = top.enter_context(tc.tile_pool(name="kt", bufs=TPC))
        v_pool = top.enter_context(tc.tile_pool(name="v", bufs=NT))
        o_pool = top.enter_context(tc.tile_pool(name="o", bufs=TPC))
        rope_pool = top.enter_context(tc.tile_pool(name="rope", bufs=2))
        p_pool = top.enter_context(tc.tile_pool(name="p", bufs=20))
        rs_pool = top.enter_context(tc.tile_pool(name="rs", bufs=1))
        rb_pool = top.enter_context(tc.tile_pool(name="rb", bufs=2))
        ob_pool = top.enter_context(tc.tile_pool(name="ob", bufs=3))
        # PSUM: pp 4 + po 2 + pr 2 = 8 banks
        pp = top.enter_context(tc.tile_pool(name="pp", bufs=4, space="PSUM"))
        ppo = top.enter_context(tc.tile_pool(name="ppo", bufs=2, space="PSUM"))
        ppr = top.enter_context(tc.tile_pool(name="ppr", bufs=2, space="PSUM"))

        l_tile = consts.tile([128, 128], FP32)
        nc.sync.dma_start(out=l_tile, in_=maskl)
        ones_t = consts.tile([128, 1], BF16)
        nc.sync.dma_start(out=ones_t, in_=ones)
        cs_t = consts.tile([128, T], FP32)   # [cos ; sin] halves
        nc.scalar.dma_start(out=cs_t, in_=cs)

        # weights on the sync DGE queue, x on the scalar DGE queue
        WQ = []
        for k in range(NK):
            w = wq_pool.tile([128, TPC * HD], BF16, tag="wq")
            nc.sync.dma_start(out=w, in_=wq[k * 128 : (k + 1) * 128, :])
            WQ.append(w)
        XG = {}

        def xdma(g):
            XG[g] = []
            for k in range(NK):
                xt = x_pool.tile([128, 512], BF16, tag="x")
                nc.scalar.dma_start(
                    out=xt,
                    in_=xT[k * 128 : (k + 1) * 128, g * 512 : (g + 1) * 512],
                )
                XG[g].append(xt)

        xdma(0)
        WK = []
        for k in range(NK):
            w = wk_pool.tile([128, TPC * HD], BF16, tag="wk")
            nc.sync.dma_start(out=w, in_=wk[k * 128 : (k + 1) * 128, :])
            WK.append(w)
        WV = []
        for k in range(NK):
            w = wv_pool.tile([128, TPC * HD], BF16, tag="wv")
            nc.sync.dma_start(out=w, in_=wv[k * 128 : (k + 1) * 128, :])
            WV.append(w)
        WP = []  # index hh*4 + m4 -> wp[hh*128:(hh+1)*128, m4*512:(m4+1)*512]
        for hh in range(TPC):
            for m4 in range(4):
                w = wp_pool.tile([128, 512], BF16, tag="wp")
                nc.sync.dma_start(
                    out=w,
                    in_=wp[hh * 128 : (hh + 1) * 128, m4 * 512 : (m4 + 1) * 512],
                )
                WP.append(w)

        QT = [qt_pool.tile([128, T], BF16, tag="qt", name=f"QT{i}") for i in range(TPC)]
        KT = [kt_pool.tile([128, T], BF16, tag="kt", name=f"KT{i}") for i in range(TPC)]
        V = [v_pool.tile([128, TPC * HD], BF16, tag="v", name=f"V{i}") for i in range(NT)]
        OT = [o_pool.tile([128, T], BF16, tag="o", name=f"OT{i}") for i in range(TPC)]

        def proj_block(g):
            """QK projection + rope + V projection for t-group g."""
            c0, c1 = g * 512, (g + 1) * 512
            for hh in range(TPC):
                for Wsrc, dstT in ((WQ, QT), (WK, KT)):
                    ps = pp.tile([128, 512], FP32, tag="pp")
                    for k in range(NK):
                        nc.tensor.matmul(
                            ps,
                            Wsrc[k][:, hh * 128 : (hh + 1) * 128],
                            XG[g][k],
                            start=(k == 0),
                            stop=(k == NK - 1),
                        )
                    # rope: out_lo = q1*c - q2*s ; out_hi = q1*s + q2*c.
                    # Half-width muls: walrus rejects SB+SB operands at
                    # different base partitions, so each mix's terms are
                    # materialized at base 0 first (PSUM+SB mul is exempt).
                    q1 = ps[0:HALF, :]
                    q2 = ps[HALF:128, :]
                    cT = cs_t[0:HALF, c0:c1]
                    sT = cs_t[HALF:128, c0:c1]
                    t1 = rope_pool.tile([HALF, 512], FP32, tag="t1")
                    t2 = rope_pool.tile([HALF, 512], FP32, tag="t2")
                    t3 = rope_pool.tile([HALF, 512], FP32, tag="t3")
                    t4 = rope_pool.tile([HALF, 512], FP32, tag="t4")
                    nc.vector.tensor_mul(t1, q1, cT)
                    nc.vector.tensor_mul(t2, q2, sT)
                    nc.vector.tensor_mul(t3, q1, sT)
                    nc.vector.tensor_mul(t4, q2, cT)
                    nc.vector.tensor_sub(dstT[hh][0:HALF, c0:c1], t1, t2)
                    nc.vector.tensor_add(dstT[hh][HALF:128, c0:c1], t3, t4)
            for tt in range(4):
                ps = pp.tile([128, 512], FP32, tag="pp")
                for k in range(NK):
                    nc.tensor.matmul(
                        ps,
                        XG[g][k][:, tt * 128 : (tt + 1) * 128],
                        WV[k],
                        start=(k == 0),
                        stop=(k == NK - 1),
                    )
                nc.scalar.copy(out=V[4 * g + tt], in_=ps)

        def attn_A_chunk(hh, g, kj, PT):
            """scores + mask + exp for one 128-key chunk."""
            c0, c1 = g * 512, (g + 1) * 512
            s0 = max(0, kj - 4 * g)
            off = s0 * 128
            w = 512 - off
            ps = pp.tile([128, 512], FP32, tag="pp")
            nc.tensor.matmul(
                ps[:, :w],
                KT[hh][:, kj * 128 : (kj + 1) * 128],
                QT[hh][:, c0 + off : c1],
                start=True,
                stop=True,
            )
            sd = kj - 4 * g
            if 0 <= sd <= 3:
                dcol = sd * 128 - off
                nc.vector.tensor_sub(
                    ps[:, dcol : dcol + 128], ps[:, dcol : dcol + 128], l_tile
                )
            pt = p_pool.tile([128, 512], BF16, tag="p")
            nc.scalar.activation(out=pt[:, :w], in_=ps[:, :w], func=EXP, scale=SCALE)
            PT.append((pt, off, w))

        def attn_B_chunk(st, kj, nch):
            hh, PT, po, rs = st
            pt, off, w = PT[kj]
            nc.tensor.matmul(rs[:, off:512], ones_t, pt[:, :w],
                             start=(kj == 0), stop=(kj == nch - 1))
            nc.tensor.matmul(po[:, off:512],
                             V[kj][:, hh * HD : (hh + 1) * HD],
                             pt[:, :w],
                             start=(kj == 0), stop=(kj == nch - 1))

        def attn_B_finish(st, g):
            hh, PT, po, rs = st
            c0, c1 = g * 512, (g + 1) * 512
            rr = rs_pool.tile([1, 512], FP32, tag="rr")
            nc.vector.tensor_copy(out=rr, in_=rs)
            ri = rs_pool.tile([1, 512], FP32, tag="ri")
            nc.vector.reciprocal_approx_fast(out=ri, in_=rr)
            rrep = rb_pool.tile([128, 512], FP32, tag="rb")
            nc.gpsimd.partition_broadcast(rrep, ri)
            nc.vector.tensor_mul(OT[hh][:, c0:c1], po, rrep)

        def attn_block(g):
            """A(h+1) interleaved chunk-by-chunk with B(h)."""
            nch = 4 * g + 4
            prev = None
            for hh in range(TPC):
                PT = []
                for kj in range(nch):
                    attn_A_chunk(hh, g, kj, PT)
                    if prev is not None:
                        attn_B_chunk(prev, kj, nch)
                if prev is not None:
                    attn_B_finish(prev, g)
                po = ppo.tile([128, 512], FP32, tag="po")
                rs = ppr.tile([1, 512], FP32, tag="rs")
                prev = (hh, PT, po, rs)
            for kj in range(nch):
                attn_B_chunk(prev, kj, nch)
            attn_B_finish(prev, g)

        def out_block(g):
            c0, c1 = g * 512, (g + 1) * 512
            for m in range(NK):
                ps = pp.tile([128, 512], FP32, tag="pp")
                for hh in range(TPC):
                    nc.tensor.matmul(
                        ps,
                        WP[hh * 4 + m // 4][:, (m % 4) * 128 : (m % 4 + 1) * 128],
                        OT[hh][:, c0:c1],
                        start=(hh == 0),
                        stop=(hh == TPC - 1),
                    )
                ob = ob_pool.tile([128, 512], FP32, tag="ob")
                nc.scalar.copy(out=ob, in_=ps)
                nc.sync.dma_start(
                    out=outT[m * 128 : (m + 1) * 128, c0:c1], in_=ob
                )

        proj_block(0)
        xdma(1)
        for g in range(NG):
            if g + 1 < NG:
                proj_block(g + 1)
                if g + 2 < NG:
                    xdma(g + 2)
            attn_block(g)
            out_block(g)


_PROGRAM = None


def _get_program():
    global _PROGRAM
    if _PROGRAM is None:
        _PROGRAM = build_program()
    return _PROGRAM


def _make_in_maps(x, cos, sin, Wqkv, Wproj):
    maskl = (np.tril(np.ones((128, 128), np.float32), -1) * 1e30).astype(np.float32)
    ones = np.ones((128, 1), dtype=BF16_NP)
    cosT = np.asarray(cos, np.float32).T   # (64, T)
    sinT = np.asarray(sin, np.float32).T
    cs = np.ascontiguousarray(np.concatenate([cosT, sinT], axis=0))
    in_maps = []
    for c in range(8):
        b, hg = c // 4, c % 4
        h0 = hg * TPC
        in_maps.append({
            "xT": np.ascontiguousarray(x[b].T.astype(BF16_NP)),
            "wq": np.ascontiguousarray(
                Wqkv[:, h0 * HD : (h0 + TPC) * HD].astype(BF16_NP)),
            "wk": np.ascontiguousarray(
                Wqkv[:, D + h0 * HD : D + (h0 + TPC) * HD].astype(BF16_NP)),
            "wv": np.ascontiguousarray(
                Wqkv[:, 2 * D + h0 * HD : 2 * D + (h0 + TPC) * HD].astype(BF16_NP)),
            "wp": np.ascontiguousarray(
                Wproj[h0 * HD : (h0 + TPC) * HD, :].astype(BF16_NP)),
            "cs": cs,
            "maskl": maskl,
            "ones": ones,
        })
    return in_maps


def _combine(results):
    outs = []
    for b in range(2):
        acc = results[4 * b]["outT"].astype(np.float32)
        for hg in range(1, 4):
            acc = acc + results[4 * b + hg]["outT"]
        outs.append(acc.T)
    return np.ascontiguousarray(np.stack(outs))


def kernel(x, cos, sin, Wqkv, Wproj):
    nc = _get_program()
    in_maps = _make_in_maps(np.asarray(x, np.float32), cos, sin,
                            np.asarray(Wqkv, np.float32), np.asarray(Wproj, np.float32))
    res = run_bass_kernel_spmd(nc, in_maps, list(range(8)))
    return _combine(res.results)


def _install_ntff_shim():
    """Provide the antenv.axon_hooks registry this container lacks, wired to
    the ctypes NTFF hook from trn_agent_boot, so trace=True works."""
    import types

    if "antenv.axon_hooks" in sys.modules:
        return
    hook = None
    try:
        from trn_agent_boot.trn_boot import _ntff_profile_via_ctypes
        hook = _ntff_profile_via_ctypes("/opt/axon/libaxon_pjrt.so")
    except Exception as e:
        print("ntff shim unavailable:", e)
    mod = types.ModuleType("antenv.axon_hooks")
    mod._hook = hook
    mod.get_axon_ntff_profile_hook = lambda: mod._hook
    mod.set_axon_ntff_profile_hook = lambda h: setattr(mod, "_hook", h)
    sys.modules["antenv.axon_hooks"] = mod
    # keep artifacts local; the bucket upload path isn't available here
    import concourse.bass_utils as bu
    bu.upload_artifacts = lambda tmpdir: tmpdir


def kernel_profiled(x, cos, sin, Wqkv, Wproj, trace_cores=None, tmpdir=None):
    nc = _get_program()
    _install_ntff_shim()
    in_maps = _make_in_maps(np.asarray(x, np.float32), cos, sin,
                            np.asarray(Wqkv, np.float32), np.asarray(Wproj, np.float32))
    res = run_bass_kernel_spmd(nc, in_maps, list(range(8)), trace=True,
                               trace_cores=trace_cores, tmpdir=tmpdir)
    return _combine(res.results), res


# revision 14
# speedup vs baseline: 1.5568x; 1.0837x over previous
"""Causal self-attention with RoPE on 8 Trainium2 NeuronCores.

Sharding: tensor-parallel over heads (4 groups of 4 heads) x data-parallel
over batch (2), one (batch, head-group) pair per core. Each core computes
its heads' QKV projection, RoPE, causal attention, and a row-slice of the
output projection; the host sums the 4 partial projections per batch.

All matmul operands are bf16 (fp32 PSUM accumulation). Q^T/K^T are
computed directly with the weight chunk stationary and x^T moving (no PE
transposes); RoPE is applied in the transposed layout, exploiting that a
fixed permutation of the head dim cancels in q.k (rotate-half instead of
interleave).

The schedule is a flat unit-interleave per round g: attention chunk-pairs
of group g (the dependency spine, with scores+exp of head h+1 interleaved
against rowsum+PV of head h), the QKV projection chains of group g+1, and
the output-projection units of group g-1, merged evenly so the PE queue
always has ready work while ACT runs exp. Score chunks are computed in
pairs into 2-bank PSUM tiles so one ACTIVATE covers 1024 columns,
halving the 352-cycle fixed cost per exp. Softmax normalization uses
reciprocal_approx_fast on the [1,512] row sums.

Hardcoded problem shape: x (2,2048,2048), Wqkv (2048,6144), Wproj
(2048,2048), cos/sin (2048,64), 16 heads, head_dim 128.
"""

import sys

sys.path.insert(0, "/opt/trn_rl_repo")

import ml_dtypes
import numpy as np

import concourse.bass as bass
import concourse.tile as tile
from concourse import bacc, mybir
from concourse.bass_utils import run_bass_kernel_spmd

B, T, D, H = 2, 2048, 2048, 16
HD, HALF = 128, 64
TPC = 4          # heads per core
NK = D // 128    # 16 contraction chunks for the projections
NG = T // 512    # 4 q/t-groups
NT = T // 128    # 16 key tiles
SCALE = float(1.0 / np.sqrt(HD))
FP32 = mybir.dt.float32
BF16 = mybir.dt.bfloat16
BF16_NP = ml_dtypes.bfloat16
EXP = mybir.ActivationFunctionType.Exp


def build_program():
    nc = bacc.Bacc("TRN2", target_bir_lowering=False, debug=False)

    xT = nc.dram_tensor("xT", [D, T], BF16, kind="ExternalInput").ap()
    wq = nc.dram_tensor("wq", [D, TPC * HD], BF16, kind="ExternalInput").ap()
    wk = nc.dram_tensor("wk", [D, TPC * HD], BF16, kind="ExternalInput").ap()
    wv = nc.dram_tensor("wv", [D, TPC * HD], BF16, kind="ExternalInput").ap()
    wp = nc.dram_tensor("wp", [TPC * HD, D], BF16, kind="ExternalInput").ap()
    cs = nc.dram_tensor("cs", [128, T], FP32, kind="ExternalInput").ap()
    maskl = nc.dram_tensor("maskl", [128, 128], FP32, kind="ExternalInput").ap()
    ones = nc.dram_tensor("ones", [128, 1], BF16, kind="ExternalInput").ap()
    outT = nc.dram_tensor("outT", [D, T], FP32, kind="ExternalOutput").ap()

    with tile.TileContext(nc) as tc:
        _kernel(tc, xT, wq, wk, wv, wp, cs, maskl, ones, outT)
    nc.compile()
    return nc


def _merge(spine, extras):
    """Spread `extras` (order-free units) evenly among `spine` units."""
    if not spine:
        return list(extras)
    out = []
    ns, ne = len(spine), len(extras)
    ei = 0
    for si, s in enumerate(spine):
        out.append(s)
        while ei < ne and (ei + 1) * ns <= (si + 1) * ne:
            out.append(extras[ei])
            ei += 1
    out.extend(extras[ei:])
    return out


def _kernel(tc, xT, wq, wk, wv, wp, cs, maskl, ones, outT):
    nc = tc.nc
    from contextlib import ExitStack

    with ExitStack() as top:
        consts = top.enter_context(tc.tile_pool(name="consts", bufs=1))
        wq_pool = top.enter_context(tc.tile_pool(name="wq", bufs=NK))
        wk_pool = top.enter_context(tc.tile_pool(name="wk", bufs=NK))
        wv_pool = top.enter_context(tc.tile_pool(name="wv", bufs=NK))
        wp_pool = top.enter_context(tc.tile_pool(name="wp", bufs=16))
        x_pool = top.enter_context(tc.tile_pool(name="x", bufs=24))
        qt_pool = top.enter_context(tc.tile_pool(name="qt", bufs=TPC))
        kt_pool = top.enter_context(tc.tile_pool(name="kt", bufs=TPC))
        v_pool = top.enter_context(tc.tile_pool(name="v", bufs=NT))
        o_pool = top.enter_context(tc.tile_pool(name="o", bufs=TPC))
        rope_pool = top.enter_context(tc.tile_pool(name="rope", bufs=1))
        p_pool = top.enter_context(tc.tile_pool(name="p", bufs=11))
        rs_pool = top.enter_context(tc.tile_pool(name="rs", bufs=1))
        rb_pool = top.enter_context(tc.tile_pool(name="rb", bufs=2))
        ob_pool = top.enter_context(tc.tile_pool(name="ob", bufs=3))
        # PSUM banks: s2 2x2 + pp 2 + po 1 + rs 1 = 8
        ps2_pool = top.enter_context(tc.tile_pool(name="ps2", bufs=2, space="PSUM"))
        pp = top.enter_context(tc.tile_pool(name="pp", bufs=2, space="PSUM"))
        ppo = top.enter_context(tc.tile_pool(name="ppo", bufs=1, space="PSUM"))
        ppr = top.enter_context(tc.tile_pool(name="ppr", bufs=1, space="PSUM"))

        l_tile = consts.tile([128, 128], FP32)
        nc.sync.dma_start(out=l_tile, in_=maskl)
        ones_t = consts.tile([128, 1], BF16)
        nc.sync.dma_start(out=ones_t, in_=ones)
        cs_t = consts.tile([128, T], FP32)   # [cos ; sin] halves
        nc.scalar.dma_start(out=cs_t, in_=cs)

        # weights on the sync DGE queue, x on the scalar DGE queue
        WQ = []
        for k in range(NK):
            w = wq_pool.tile([128, TPC * HD], BF16, tag="wq")
            nc.sync.dma_start(out=w, in_=wq[k * 128 : (k + 1) * 128, :])
            WQ.append(w)
        XG = {}

        def xdma(g):
            XG[g] = []
            for k in range(NK):
                xt = x_pool.tile([128, 512], BF16, tag="x")
                nc.scalar.dma_start(
                    out=xt,
                    in_=xT[k * 128 : (k + 1) * 128, g * 512 : (g + 1) * 512],
                )
                XG[g].append(xt)

        xdma(0)
        WK = []
        for k in range(NK):
            w = wk_pool.tile([128, TPC * HD], BF16, tag="wk")
            nc.sync.dma_start(out=w, in_=wk[k * 128 : (k + 1) * 128, :])
            WK.append(w)
        WV = []
        for k in range(NK):
            w = wv_pool.tile([128, TPC * HD], BF16, tag="wv")
            nc.sync.dma_start(out=w, in_=wv[k * 128 : (k + 1) * 128, :])
            WV.append(w)
        WP = []  # index hh*4 + m4 -> wp[hh*128:(hh+1)*128, m4*512:(m4+1)*512]
        for hh in range(TPC):
            for m4 in range(4):
                w = wp_pool.tile([128, 512], BF16, tag="wp")
                nc.sync.dma_start(
                    out=w,
                    in_=wp[hh * 128 : (hh + 1) * 128, m4 * 512 : (m4 + 1) * 512],
                )
                WP.append(w)

        QT = [qt_pool.tile([128, T], BF16, tag="qt", name=f"QT{i}") for i in range(TPC)]
        KT = [kt_pool.tile([128, T], BF16, tag="kt", name=f"KT{i}") for i in range(TPC)]
        V = [v_pool.tile([128, TPC * HD], BF16, tag="v", name=f"V{i}") for i in range(NT)]
        OT = [o_pool.tile([128, T], BF16, tag="o", name=f"OT{i}") for i in range(TPC)]

        # ---------------- unit bodies ----------------

        def qk_chain(g, hh, Wsrc, dstT):
            c0, c1 = g * 512, (g + 1) * 512
            ps = pp.tile([128, 512], FP32, tag="pp")
            for k in range(NK):
                nc.tensor.matmul(
                    ps,
                    Wsrc[k][:, hh * 128 : (hh + 1) * 128],
                    XG[g][k],
                    start=(k == 0),
                    stop=(k == NK - 1),
                )
            # rope: out_lo = q1*c - q2*s ; out_hi = q1*s + q2*c (terms
            # materialized at base 0: SB+SB operands must share a base).
            q1 = ps[0:HALF, :]
            q2 = ps[HALF:128, :]
            cT = cs_t[0:HALF, c0:c1]
            sT = cs_t[HALF:128, c0:c1]
            t1 = rope_pool.tile([HALF, 512], FP32, tag="t1")
            t2 = rope_pool.tile([HALF, 512], FP32, tag="t2")
            t3 = rope_pool.tile([HALF, 512], FP32, tag="t3")
            t4 = rope_pool.tile([HALF, 512], FP32, tag="t4")
            nc.vector.tensor_mul(t1, q1, cT)
            nc.vector.tensor_mul(t2, q2, sT)
            nc.vector.tensor_mul(t3, q1, sT)
            nc.vector.tensor_mul(t4, q2, cT)
            nc.vector.tensor_sub(dstT[hh][0:HALF, c0:c1], t1, t2)
            nc.vector.tensor_add(dstT[hh][HALF:128, c0:c1], t3, t4)

        def v_chain(g, tt):
            ps = pp.tile([128, 512], FP32, tag="pp")
            for k in range(NK):
                nc.tensor.matmul(
                    ps,
                    XG[g][k][:, tt * 128 : (tt + 1) * 128],
                    WV[k],
                    start=(k == 0),
                    stop=(k == NK - 1),
                )
            nc.scalar.copy(out=V[4 * g + tt], in_=ps)

        def a2_unit(hh, g, jp, PT2):
            """Two score chunks into one 2-bank PSUM tile + one exp."""
            c0, c1 = g * 512, (g + 1) * 512
            ps2 = ps2_pool.tile([128, 1024], FP32, tag="s2")
            pt2 = p_pool.tile([128, 1024], BF16, tag="p")
            ws = []
            for half in range(2):
                kj = 2 * jp + half
                s0 = max(0, kj - 4 * g)
                off = s0 * 128
                w = 512 - off
                base = half * 512
                nc.tensor.matmul(
                    ps2[:, base : base + w],
                    KT[hh][:, kj * 128 : (kj + 1) * 128],
                    QT[hh][:, c0 + off : c1],
                    start=True,
                    stop=True,
                )
                sd = kj - 4 * g
                if 0 <= sd <= 3:
                    dcol = sd * 128 - off
                    nc.vector.tensor_sub(
                        ps2[:, base + dcol : base + dcol + 128],
                        ps2[:, base + dcol : base + dcol + 128],
                        l_tile,
                    )
                ws.append((off, w))
            w1 = ws[1][1]
            nc.scalar.activation(out=pt2[:, : 512 + w1], in_=ps2[:, : 512 + w1],
                                 func=EXP, scale=SCALE)
            PT2.append((pt2, ws))

        def b_unit(st, kj):
            hh, g, nch, PT2 = st["hh"], st["g"], st["nch"], st["PT2"]
            if st["po"] is None:
                st["po"] = ppo.tile([128, 512], FP32, tag="po",
                                    name=f"po{g}_{hh}")
                st["rs"] = ppr.tile([1, 512], FP32, tag="rs",
                                    name=f"rsum{g}_{hh}")
            po, rs = st["po"], st["rs"]
            pt2, ws = PT2[kj // 2]
            off, w = ws[kj % 2]
            base = (kj % 2) * 512
            nc.tensor.matmul(rs[:, off:512], ones_t, pt2[:, base : base + w],
                             start=(kj == 0), stop=(kj == nch - 1))
            nc.tensor.matmul(po[:, off:512],
                             V[kj][:, hh * HD : (hh + 1) * HD],
                             pt2[:, base : base + w],
                             start=(kj == 0), stop=(kj == nch - 1))

        def b_finish(st):
            hh, g = st["hh"], st["g"]
            c0, c1 = g * 512, (g + 1) * 512
            rr = rs_pool.tile([1, 512], FP32, tag="rr")
            nc.vector.tensor_copy(out=rr, in_=st["rs"])
            ri = rs_pool.tile([1, 512], FP32, tag="ri")
            nc.vector.reciprocal_approx_fast(out=ri, in_=rr)
            rrep = rb_pool.tile([128, 512], FP32, tag="rb")
            nc.gpsimd.partition_broadcast(rrep, ri)
            nc.vector.tensor_mul(OT[hh][:, c0:c1], st["po"], rrep)

        def out_unit(g, m):
            c0, c1 = g * 512, (g + 1) * 512
            ps = pp.tile([128, 512], FP32, tag="pp")
            for hh in range(TPC):
                nc.tensor.matmul(
                    ps,
                    WP[hh * 4 + m // 4][:, (m % 4) * 128 : (m % 4 + 1) * 128],
                    OT[hh][:, c0:c1],
                    start=(hh == 0),
                    stop=(hh == TPC - 1),
                )
            ob = ob_pool.tile([128, 512], FP32, tag="ob")
            nc.vector.tensor_copy(out=ob, in_=ps)
            nc.sync.dma_start(out=outT[m * 128 : (m + 1) * 128, c0:c1], in_=ob)

        # ---------------- unit lists ----------------

        def proj_units(g):
            us = []
            for hh in range(TPC):
                for Wsrc, dstT in ((WQ, QT), (WK, KT)):
                    us.append(lambda g=g, hh=hh, Wsrc=Wsrc, dstT=dstT:
                              qk_chain(g, hh, Wsrc, dstT))
            for tt in range(4):
                us.append(lambda g=g, tt=tt: v_chain(g, tt))
            return us

        def attn_spine(g):
            nch = 4 * g + 4
            npair = nch // 2
            units = []
            prev = None
            for hh in range(TPC):
                st = {"hh": hh, "g": g, "nch": nch, "PT2": [], "po": None,
                      "rs": None}
                for jp in range(npair):
                    units.append(lambda hh=hh, g=g, jp=jp, PT2=st["PT2"]:
                                 a2_unit(hh, g, jp, PT2))
                    if prev is not None:
                        for t in range(2):
                            units.append(lambda prev=prev, kj=2 * jp + t:
                                         b_unit(prev, kj))
                if prev is not None:
                    units.append(lambda prev=prev: b_finish(prev))
                prev = st
            for kj in range(nch):
                units.append(lambda prev=prev, kj=kj: b_unit(prev, kj))
            units.append(lambda prev=prev: b_finish(prev))
            return units

        def out_units(g):
            return [lambda g=g, m=m: out_unit(g, m) for m in range(NK)]

        # ---------------- schedule ----------------

        xdma(1)
        for u in proj_units(0):
            u()
        for g in range(NG):
            spine = attn_spine(g)
            extras = []
            if g >= 1:
                extras += out_units(g - 1)   # ready immediately: cover for
            if g + 1 < NG:                   # x(g+1) DMA before proj chains
                extras += proj_units(g + 1)
            for u in _merge(spine, extras):
                u()
            if g + 2 < NG:
                xdma(g + 2)
        for u in out_units(NG - 1):
            u()


_PROGRAM = None


def _get_program():
    global _PROGRAM
    if _PROGRAM is None:
        _PROGRAM = build_program()
    return _PROGRAM


def _make_in_maps(x, cos, sin, Wqkv, Wproj):
    maskl = (np.tril(np.ones((128, 128), np.float32), -1) * 1e30).astype(np.float32)
    ones = np.ones((128, 1), dtype=BF16_NP)
    cosT = np.asarray(cos, np.float32).T   # (64, T)
    sinT = np.asarray(sin, np.float32).T
    cs = np.ascontiguousarray(np.concatenate([cosT, sinT], axis=0))
    in_maps = []
    for c in range(8):
        b, hg = c // 4, c % 4
        h0 = hg * TPC
        in_maps.append({
            "xT": np.ascontiguousarray(x[b].T.astype(BF16_NP)),
            "wq": np.ascontiguousarray(
                Wqkv[:, h0 * HD : (h0 + TPC) * HD].astype(BF16_NP)),
            "wk": np.ascontiguousarray(
                Wqkv[:, D + h0 * HD : D + (h0 + TPC) * HD].astype(BF16_NP)),
            "wv": np.ascontiguousarray(
                Wqkv[:, 2 * D + h0 * HD : 2 * D + (h0 + TPC) * HD].astype(BF16_NP)),
            "wp": np.ascontiguousarray(
                Wproj[h0 * HD : (h0 + TPC) * HD, :].astype(BF16_NP)),
            "cs": cs,
            "maskl": maskl,
            "ones": ones,
        })
    return in_maps


def _combine(results):
    outs = []
    for b in range(2):
        acc = results[4 * b]["outT"].astype(np.float32)
        for hg in range(1, 4):
            acc = acc + results[4 * b + hg]["outT"]
        outs.append(acc.T)
    return np.ascontiguousarray(np.stack(outs))


def kernel(x, cos, sin, Wqkv, Wproj):
    nc = _get_program()
    in_maps = _make_in_maps(np.asarray(x, np.float32), cos, sin,
                            np.asarray(Wqkv, np.float32), np.asarray(Wproj, np.float32))
    res = run_bass_kernel_spmd(nc, in_maps, list(range(8)))
    return _combine(res.results)


def _install_ntff_shim():
    """Provide the antenv.axon_hooks registry this container lacks, wired to
    the ctypes NTFF hook from trn_agent_boot, so trace=True works."""
    import types

    if "antenv.axon_hooks" in sys.modules:
        return
    hook = None
    try:
        from trn_agent_boot.trn_boot import _ntff_profile_via_ctypes
        hook = _ntff_profile_via_ctypes("/opt/axon/libaxon_pjrt.so")
    except Exception as e:
        print("ntff shim unavailable:", e)
    mod = types.ModuleType("antenv.axon_hooks")
    mod._hook = hook
    mod.get_axon_ntff_profile_hook = lambda: mod._hook
    mod.set_axon_ntff_profile_hook = lambda h: setattr(mod, "_hook", h)
    sys.modules["antenv.axon_hooks"] = mod
    # keep artifacts local; the bucket upload path isn't available here
    import concourse.bass_utils as bu
    bu.upload_artifacts = lambda tmpdir: tmpdir


def kernel_profiled(x, cos, sin, Wqkv, Wproj, trace_cores=None, tmpdir=None):
    nc = _get_program()
    _install_ntff_shim()
    in_maps = _make_in_maps(np.asarray(x, np.float32), cos, sin,
                            np.asarray(Wqkv, np.float32), np.asarray(Wproj, np.float32))
    res = run_bass_kernel_spmd(nc, in_maps, list(range(8)), trace=True,
                               trace_cores=trace_cores, tmpdir=tmpdir)
    return _combine(res.results), res


# revision 18
# speedup vs baseline: 1.5656x; 1.0056x over previous
"""Causal self-attention with RoPE on 8 Trainium2 NeuronCores.

Sharding: tensor-parallel over heads (4 groups of 4 heads) x data-parallel
over batch (2), one (batch, head-group) pair per core. Each core computes
its heads' QKV projection, RoPE, causal attention, and a row-slice of the
output projection; the host sums the 4 partial projections per batch.

All matmul operands are bf16 (fp32 PSUM accumulation). Q^T/K^T are
computed directly with the weight chunk stationary and x^T moving (no PE
transposes); RoPE is applied in the transposed layout, exploiting that a
fixed permutation of the head dim cancels in q.k (rotate-half instead of
interleave).

The schedule is a flat unit-interleave per round g: attention chunk-pairs
of group g (the dependency spine, with scores+exp of head h+1 interleaved
against rowsum+PV of head h), the QKV projection chains of group g+1, and
the output-projection units of group g-1, merged evenly so the PE queue
always has ready work while ACT runs exp. Score chunks are computed in
pairs into 2-bank PSUM tiles so one ACTIVATE covers 1024 columns,
halving the 352-cycle fixed cost per exp. Softmax normalization uses
reciprocal_approx_fast on the [1,512] row sums.

Hardcoded problem shape: x (2,2048,2048), Wqkv (2048,6144), Wproj
(2048,2048), cos/sin (2048,64), 16 heads, head_dim 128.
"""

import sys

sys.path.insert(0, "/opt/trn_rl_repo")

import ml_dtypes
import numpy as np

import concourse.bass as bass
import concourse.tile as tile
from concourse import bacc, mybir
from concourse.bass_utils import run_bass_kernel_spmd

B, T, D, H = 2, 2048, 2048, 16
HD, HALF = 128, 64
TPC = 4          # heads per core
NK = D // 128    # 16 contraction chunks for the projections
NG = T // 512    # 4 q/t-groups
NT = T // 128    # 16 key tiles
SCALE = float(1.0 / np.sqrt(HD))
FP32 = mybir.dt.float32
BF16 = mybir.dt.bfloat16
BF16_NP = ml_dtypes.bfloat16
EXP = mybir.ActivationFunctionType.Exp


def build_program():
    nc = bacc.Bacc("TRN2", target_bir_lowering=False, debug=False)

    xT = nc.dram_tensor("xT", [D, T], BF16, kind="ExternalInput").ap()
    wq = nc.dram_tensor("wq", [D, TPC * HD], BF16, kind="ExternalInput").ap()
    wk = nc.dram_tensor("wk", [D, TPC * HD], BF16, kind="ExternalInput").ap()
    wv = nc.dram_tensor("wv", [D, TPC * HD], BF16, kind="ExternalInput").ap()
    wp = nc.dram_tensor("wp", [TPC * HD, D], BF16, kind="ExternalInput").ap()
    cs = nc.dram_tensor("cs", [128, T], FP32, kind="ExternalInput").ap()
    maskl = nc.dram_tensor("maskl", [128, 128], FP32, kind="ExternalInput").ap()
    ones = nc.dram_tensor("ones", [128, 1], BF16, kind="ExternalInput").ap()
    outT = nc.dram_tensor("outT", [D, T], BF16, kind="ExternalOutput").ap()

    with tile.TileContext(nc) as tc:
        _kernel(tc, xT, wq, wk, wv, wp, cs, maskl, ones, outT)
    nc.compile()
    return nc


def _merge(spine, extras):
    """Spread `extras` (order-free units) evenly among `spine` units."""
    if not spine:
        return list(extras)
    out = []
    ns, ne = len(spine), len(extras)
    ei = 0
    for si, s in enumerate(spine):
        out.append(s)
        while ei < ne and (ei + 1) * ns <= (si + 1) * ne:
            out.append(extras[ei])
            ei += 1
    out.extend(extras[ei:])
    return out


def _kernel(tc, xT, wq, wk, wv, wp, cs, maskl, ones, outT):
    nc = tc.nc
    from contextlib import ExitStack

    with ExitStack() as top:
        consts = top.enter_context(tc.tile_pool(name="consts", bufs=1))
        wq_pool = top.enter_context(tc.tile_pool(name="wq", bufs=NK))
        wk_pool = top.enter_context(tc.tile_pool(name="wk", bufs=NK))
        wv_pool = top.enter_context(tc.tile_pool(name="wv", bufs=NK))
        wp_pool = top.enter_context(tc.tile_pool(name="wp", bufs=16))
        x_pool = top.enter_context(tc.tile_pool(name="x", bufs=26))
        qt_pool = top.enter_context(tc.tile_pool(name="qt", bufs=TPC))
        kt_pool = top.enter_context(tc.tile_pool(name="kt", bufs=TPC))
        v_pool = top.enter_context(tc.tile_pool(name="v", bufs=NT))
        o_pool = top.enter_context(tc.tile_pool(name="o", bufs=TPC))
        rope_pool = top.enter_context(tc.tile_pool(name="rope", bufs=1))
        p_pool = top.enter_context(tc.tile_pool(name="p", bufs=11))
        rs_pool = top.enter_context(tc.tile_pool(name="rs", bufs=1))
        rb_pool = top.enter_context(tc.tile_pool(name="rb", bufs=2))
        ob_pool = top.enter_context(tc.tile_pool(name="ob", bufs=3))
        # PSUM banks: s2 2x2 + pp 2 + po 1 + rs 1 = 8
        ps2_pool = top.enter_context(tc.tile_pool(name="ps2", bufs=2, space="PSUM"))
        pp = top.enter_context(tc.tile_pool(name="pp", bufs=2, space="PSUM"))
        ppo = top.enter_context(tc.tile_pool(name="ppo", bufs=1, space="PSUM"))
        ppr = top.enter_context(tc.tile_pool(name="ppr", bufs=1, space="PSUM"))

        l_tile = consts.tile([128, 128], FP32)
        nc.sync.dma_start(out=l_tile, in_=maskl)
        ones_t = consts.tile([128, 1], BF16)
        nc.sync.dma_start(out=ones_t, in_=ones)
        cs_t = consts.tile([128, T], FP32)   # [cos ; sin] halves
        nc.scalar.dma_start(out=cs_t, in_=cs)

        # weights on the sync DGE queue, x on the scalar DGE queue
        WQ = []
        for k in range(NK):
            w = wq_pool.tile([128, TPC * HD], BF16, tag="wq")
            nc.sync.dma_start(out=w, in_=wq[k * 128 : (k + 1) * 128, :])
            WQ.append(w)
        XG = {}

        def xdma(g):
            XG[g] = []
            for k in range(NK):
                xt = x_pool.tile([128, 512], BF16, tag="x")
                nc.scalar.dma_start(
                    out=xt,
                    in_=xT[k * 128 : (k + 1) * 128, g * 512 : (g + 1) * 512],
                )
                XG[g].append(xt)

        xdma(0)
        WK = []
        for k in range(NK):
            w = wk_pool.tile([128, TPC * HD], BF16, tag="wk")
            nc.sync.dma_start(out=w, in_=wk[k * 128 : (k + 1) * 128, :])
            WK.append(w)
        WV = []
        for k in range(NK):
            w = wv_pool.tile([128, TPC * HD], BF16, tag="wv")
            nc.sync.dma_start(out=w, in_=wv[k * 128 : (k + 1) * 128, :])
            WV.append(w)
        WP = []  # index hh*4 + m4 -> wp[hh*128:(hh+1)*128, m4*512:(m4+1)*512]
        for hh in range(TPC):
            for m4 in range(4):
                w = wp_pool.tile([128, 512], BF16, tag="wp")
                nc.sync.dma_start(
                    out=w,
                    in_=wp[hh * 128 : (hh + 1) * 128, m4 * 512 : (m4 + 1) * 512],
                )
                WP.append(w)

        QT = [qt_pool.tile([128, T], BF16, tag="qt", name=f"QT{i}") for i in range(TPC)]
        KT = [kt_pool.tile([128, T], BF16, tag="kt", name=f"KT{i}") for i in range(TPC)]
        V = [v_pool.tile([128, TPC * HD], BF16, tag="v", name=f"V{i}") for i in range(NT)]
        OT = [o_pool.tile([128, T], BF16, tag="o", name=f"OT{i}") for i in range(TPC)]

        # ---------------- unit bodies ----------------

        def qk_chain(g, hh, Wsrc, dstT):
            c0, c1 = g * 512, (g + 1) * 512
            ps = pp.tile([128, 512], FP32, tag="pp")
            for k in range(NK):
                nc.tensor.matmul(
                    ps,
                    Wsrc[k][:, hh * 128 : (hh + 1) * 128],
                    XG[g][k],
                    start=(k == 0),
                    stop=(k == NK - 1),
                )
            # rope: out_lo = q1*c - q2*s ; out_hi = q1*s + q2*c (terms
            # materialized at base 0: SB+SB operands must share a base).
            q1 = ps[0:HALF, :]
            q2 = ps[HALF:128, :]
            cT = cs_t[0:HALF, c0:c1]
            sT = cs_t[HALF:128, c0:c1]
            t1 = rope_pool.tile([HALF, 512], FP32, tag="t1")
            t2 = rope_pool.tile([HALF, 512], FP32, tag="t2")
            t3 = rope_pool.tile([HALF, 512], FP32, tag="t3")
            t4 = rope_pool.tile([HALF, 512], FP32, tag="t4")
            nc.vector.tensor_mul(t1, q1, cT)
            nc.vector.tensor_mul(t2, q2, sT)
            nc.vector.tensor_mul(t3, q1, sT)
            nc.vector.tensor_mul(t4, q2, cT)
            nc.vector.tensor_sub(dstT[hh][0:HALF, c0:c1], t1, t2)
            nc.vector.tensor_add(dstT[hh][HALF:128, c0:c1], t3, t4)

        def v_chain(g, tt):
            ps = pp.tile([128, 512], FP32, tag="pp")
            for k in range(NK):
                nc.tensor.matmul(
                    ps,
                    XG[g][k][:, tt * 128 : (tt + 1) * 128],
                    WV[k],
                    start=(k == 0),
                    stop=(k == NK - 1),
                )
            nc.scalar.copy(out=V[4 * g + tt], in_=ps)

        def a2_unit(hh, g, jp, PT2):
            """Two score chunks into one 2-bank PSUM tile + one exp."""
            c0, c1 = g * 512, (g + 1) * 512
            ps2 = ps2_pool.tile([128, 1024], FP32, tag="s2")
            pt2 = p_pool.tile([128, 1024], BF16, tag="p")
            ws = []
            for half in range(2):
                kj = 2 * jp + half
                s0 = max(0, kj - 4 * g)
                off = s0 * 128
                w = 512 - off
                base = half * 512
                nc.tensor.matmul(
                    ps2[:, base : base + w],
                    KT[hh][:, kj * 128 : (kj + 1) * 128],
                    QT[hh][:, c0 + off : c1],
                    start=True,
                    stop=True,
                )
                sd = kj - 4 * g
                if 0 <= sd <= 3:
                    dcol = sd * 128 - off
                    nc.vector.tensor_sub(
                        ps2[:, base + dcol : base + dcol + 128],
                        ps2[:, base + dcol : base + dcol + 128],
                        l_tile,
                    )
                ws.append((off, w))
            w1 = ws[1][1]
            nc.scalar.activation(out=pt2[:, : 512 + w1], in_=ps2[:, : 512 + w1],
                                 func=EXP, scale=SCALE)
            PT2.append((pt2, ws))

        def b_unit(st, kj):
            hh, g, nch, PT2 = st["hh"], st["g"], st["nch"], st["PT2"]
            if st["po"] is None:
                st["po"] = ppo.tile([128, 512], FP32, tag="po",
                                    name=f"po{g}_{hh}")
                st["rs"] = ppr.tile([1, 512], FP32, tag="rs",
                                    name=f"rsum{g}_{hh}")
            po, rs = st["po"], st["rs"]
            pt2, ws = PT2[kj // 2]
            off, w = ws[kj % 2]
            base = (kj % 2) * 512
            nc.tensor.matmul(rs[:, off:512], ones_t, pt2[:, base : base + w],
                             start=(kj == 0), stop=(kj == nch - 1))
            nc.tensor.matmul(po[:, off:512],
                             V[kj][:, hh * HD : (hh + 1) * HD],
                             pt2[:, base : base + w],
                             start=(kj == 0), stop=(kj == nch - 1))

        def b_finish(st):
            hh, g = st["hh"], st["g"]
            c0, c1 = g * 512, (g + 1) * 512
            rr = rs_pool.tile([1, 512], FP32, tag="rr")
            nc.vector.tensor_copy(out=rr, in_=st["rs"])
            ri = rs_pool.tile([1, 512], FP32, tag="ri")
            nc.vector.reciprocal_approx_fast(out=ri, in_=rr)
            rrep = rb_pool.tile([128, 512], FP32, tag="rb")
            nc.gpsimd.partition_broadcast(rrep, ri)
            nc.vector.tensor_mul(OT[hh][:, c0:c1], st["po"], rrep)

        def out_unit(g, m):
            c0, c1 = g * 512, (g + 1) * 512
            ps = pp.tile([128, 512], FP32, tag="pp")
            for hh in range(TPC):
                nc.tensor.matmul(
                    ps,
                    WP[hh * 4 + m // 4][:, (m % 4) * 128 : (m % 4 + 1) * 128],
                    OT[hh][:, c0:c1],
                    start=(hh == 0),
                    stop=(hh == TPC - 1),
                )
            ob = ob_pool.tile([128, 512], BF16, tag="ob")
            nc.scalar.copy(out=ob, in_=ps)
            nc.sync.dma_start(out=outT[m * 128 : (m + 1) * 128, c0:c1], in_=ob)

        # ---------------- unit lists ----------------

        def proj_units(g):
            us = []
            for hh in range(TPC):
                for Wsrc, dstT in ((WQ, QT), (WK, KT)):
                    us.append(lambda g=g, hh=hh, Wsrc=Wsrc, dstT=dstT:
                              qk_chain(g, hh, Wsrc, dstT))
            for tt in range(4):
                us.append(lambda g=g, tt=tt: v_chain(g, tt))
            return us

        def attn_spine(g):
            nch = 4 * g + 4
            npair = nch // 2
            units = []
            prev = None
            for hh in range(TPC):
                st = {"hh": hh, "g": g, "nch": nch, "PT2": [], "po": None,
                      "rs": None}
                for jp in range(npair):
                    units.append(lambda hh=hh, g=g, jp=jp, PT2=st["PT2"]:
                                 a2_unit(hh, g, jp, PT2))
                    if prev is not None:
                        for t in range(2):
                            units.append(lambda prev=prev, kj=2 * jp + t:
                                         b_unit(prev, kj))
                if prev is not None:
                    units.append(lambda prev=prev: b_finish(prev))
                prev = st
            for kj in range(nch):
                units.append(lambda prev=prev, kj=kj: b_unit(prev, kj))
            units.append(lambda prev=prev: b_finish(prev))
            return units

        def out_units(g):
            return [lambda g=g, m=m: out_unit(g, m) for m in range(NK)]

        # ---------------- schedule ----------------

        xdma(1)
        for u in proj_units(0):
            u()
        for g in range(NG):
            spine = attn_spine(g)
            extras = []
            if g >= 1:
                extras += out_units(g - 1)   # ready immediately: cover for
            if g + 1 < NG:                   # x(g+1) DMA before proj chains
                extras += proj_units(g + 1)
            for u in _merge(spine, extras):
                u()
            if g + 2 < NG:
                xdma(g + 2)
        for u in out_units(NG - 1):
            u()


_PROGRAM = None


def _get_program():
    global _PROGRAM
    if _PROGRAM is None:
        _PROGRAM = build_program()
    return _PROGRAM


def _make_in_maps(x, cos, sin, Wqkv, Wproj):
    maskl = (np.tril(np.ones((128, 128), np.float32), -1) * 1e30).astype(np.float32)
    ones = np.ones((128, 1), dtype=BF16_NP)
    cosT = np.asarray(cos, np.float32).T   # (64, T)
    sinT = np.asarray(sin, np.float32).T
    cs = np.ascontiguousarray(np.concatenate([cosT, sinT], axis=0))
    in_maps = []
    for c in range(8):
        b, hg = c // 4, c % 4
        h0 = hg * TPC
        in_maps.append({
            "xT": np.ascontiguousarray(x[b].T.astype(BF16_NP)),
            "wq": np.ascontiguousarray(
                Wqkv[:, h0 * HD : (h0 + TPC) * HD].astype(BF16_NP)),
            "wk": np.ascontiguousarray(
                Wqkv[:, D + h0 * HD : D + (h0 + TPC) * HD].astype(BF16_NP)),
            "wv": np.ascontiguousarray(
                Wqkv[:, 2 * D + h0 * HD : 2 * D + (h0 + TPC) * HD].astype(BF16_NP)),
            "wp": np.ascontiguousarray(
                Wproj[h0 * HD : (h0 + TPC) * HD, :].astype(BF16_NP)),
            "cs": cs,
            "maskl": maskl,
            "ones": ones,
        })
    return in_maps


def _combine(results):
    outs = []
    for b in range(2):
        acc = results[4 * b]["outT"].astype(np.float32)
        for hg in range(1, 4):
            acc = acc + results[4 * b + hg]["outT"].astype(np.float32)
        outs.append(acc.T)
    return np.ascontiguousarray(np.stack(outs))


def kernel(x, cos, sin, Wqkv, Wproj):
    nc = _get_program()
    in_maps = _make_in_maps(np.asarray(x, np.float32), cos, sin,
                            np.asarray(Wqkv, np.float32), np.asarray(Wproj, np.float32))
    res = run_bass_kernel_spmd(nc, in_maps, list(range(8)))
    return _combine(res.results)


def _install_ntff_shim():
    """Provide the antenv.axon_hooks registry this container lacks, wired to
    the ctypes NTFF hook from trn_agent_boot, so trace=True works."""
    import types

    if "antenv.axon_hooks" in sys.modules:
        return
    hook = None
    try:
        from trn_agent_boot.trn_boot import _ntff_profile_via_ctypes
        hook = _ntff_profile_via_ctypes("/opt/axon/libaxon_pjrt.so")
    except Exception as e:
        print("ntff shim unavailable:", e)
    mod = types.ModuleType("antenv.axon_hooks")
    mod._hook = hook
    mod.get_axon_ntff_profile_hook = lambda: mod._hook
    mod.set_axon_ntff_profile_hook = lambda h: setattr(mod, "_hook", h)
    sys.modules["antenv.axon_hooks"] = mod
    # keep artifacts local; the bucket upload path isn't available here
    import concourse.bass_utils as bu
    bu.upload_artifacts = lambda tmpdir: tmpdir


def kernel_profiled(x, cos, sin, Wqkv, Wproj, trace_cores=None, tmpdir=None):
    nc = _get_program()
    _install_ntff_shim()
    in_maps = _make_in_maps(np.asarray(x, np.float32), cos, sin,
                            np.asarray(Wqkv, np.float32), np.asarray(Wproj, np.float32))
    res = run_bass_kernel_spmd(nc, in_maps, list(range(8)), trace=True,
                               trace_cores=trace_cores, tmpdir=tmpdir)
    return _combine(res.results), res


# revision 22
# speedup vs baseline: 1.6055x; 1.0255x over previous
"""Causal self-attention with RoPE on 8 Trainium2 NeuronCores.

Sharding: tensor-parallel over heads (4 groups of 4 heads) x data-parallel
over batch (2), one (batch, head-group) pair per core. Each core computes
its heads' QKV projection, RoPE, causal attention, and a row-slice of the
output projection; the host sums the 4 partial projections per batch.

All matmul operands are bf16 (fp32 PSUM accumulation). Q^T/K^T are
computed directly with the weight chunk stationary and x^T moving (no PE
transposes); RoPE is applied in the transposed layout, exploiting that a
fixed permutation of the head dim cancels in q.k (rotate-half instead of
interleave).

The schedule is a flat unit-interleave per round g: attention chunk-pairs
of group g (the dependency spine, with scores+exp of head h+1 interleaved
against rowsum+PV of head h), the QKV projection chains of group g+1, and
the output-projection units of group g-1, merged evenly so the PE queue
always has ready work while ACT runs exp. Score chunks are computed in
pairs into 2-bank PSUM tiles so one ACTIVATE covers 1024 columns,
halving the 352-cycle fixed cost per exp. Softmax normalization uses
reciprocal_approx_fast on the [1,512] row sums.

Hardcoded problem shape: x (2,2048,2048), Wqkv (2048,6144), Wproj
(2048,2048), cos/sin (2048,64), 16 heads, head_dim 128.
"""

import sys

sys.path.insert(0, "/opt/trn_rl_repo")

import ml_dtypes
import numpy as np

import concourse.bass as bass
import concourse.tile as tile
from concourse import bacc, mybir
from concourse.bass_utils import run_bass_kernel_spmd

B, T, D, H = 2, 2048, 2048, 16
HD, HALF = 128, 64
TPC = 4          # heads per core
NK = D // 128    # 16 contraction chunks for the projections
NG = T // 512    # 4 q/t-groups
NT = T // 128    # 16 key tiles
SCALE = float(1.0 / np.sqrt(HD))
FP32 = mybir.dt.float32
BF16 = mybir.dt.bfloat16
BF16_NP = ml_dtypes.bfloat16
EXP = mybir.ActivationFunctionType.Exp


def build_program():
    nc = bacc.Bacc("TRN2", target_bir_lowering=False, debug=False)

    xT = nc.dram_tensor("xT", [D, T], BF16, kind="ExternalInput").ap()
    wq = nc.dram_tensor("wq", [D, TPC * HD], BF16, kind="ExternalInput").ap()
    wk = nc.dram_tensor("wk", [D, TPC * HD], BF16, kind="ExternalInput").ap()
    wv = nc.dram_tensor("wv", [D, TPC * HD], BF16, kind="ExternalInput").ap()
    wp = nc.dram_tensor("wp", [TPC * HD, D], BF16, kind="ExternalInput").ap()
    cs = nc.dram_tensor("cs", [128, T], FP32, kind="ExternalInput").ap()
    maskl = nc.dram_tensor("maskl", [128, 128], FP32, kind="ExternalInput").ap()
    ones = nc.dram_tensor("ones", [128, 1], BF16, kind="ExternalInput").ap()
    outT = nc.dram_tensor("outT", [D, T], BF16, kind="ExternalOutput").ap()

    with tile.TileContext(nc) as tc:
        _kernel(tc, xT, wq, wk, wv, wp, cs, maskl, ones, outT)
    nc.compile()
    return nc


def _merge(spine, extras):
    """Spread `extras` (order-free units) evenly among `spine` units."""
    if not spine:
        return list(extras)
    out = []
    ns, ne = len(spine), len(extras)
    ei = 0
    for si, s in enumerate(spine):
        out.append(s)
        while ei < ne and (ei + 1) * ns <= (si + 1) * ne:
            out.append(extras[ei])
            ei += 1
    out.extend(extras[ei:])
    return out


def _kernel(tc, xT, wq, wk, wv, wp, cs, maskl, ones, outT):
    nc = tc.nc
    from contextlib import ExitStack

    with ExitStack() as top:
        consts = top.enter_context(tc.tile_pool(name="consts", bufs=1))
        wq_pool = top.enter_context(tc.tile_pool(name="wq", bufs=NK))
        wk_pool = top.enter_context(tc.tile_pool(name="wk", bufs=NK))
        wv_pool = top.enter_context(tc.tile_pool(name="wv", bufs=NK))
        wp_pool = top.enter_context(tc.tile_pool(name="wp", bufs=16))
        x_pool = top.enter_context(tc.tile_pool(name="x", bufs=26))
        qt_pool = top.enter_context(tc.tile_pool(name="qt", bufs=TPC))
        kt_pool = top.enter_context(tc.tile_pool(name="kt", bufs=TPC))
        v_pool = top.enter_context(tc.tile_pool(name="v", bufs=NT))
        o_pool = top.enter_context(tc.tile_pool(name="o", bufs=TPC))
        rope_pool = top.enter_context(tc.tile_pool(name="rope", bufs=1))
        p_pool = top.enter_context(tc.tile_pool(name="p", bufs=11))
        rs_pool = top.enter_context(tc.tile_pool(name="rs", bufs=1))
        rb_pool = top.enter_context(tc.tile_pool(name="rb", bufs=2))
        ob_pool = top.enter_context(tc.tile_pool(name="ob", bufs=3))
        # PSUM banks: s2 2x2 + pp 2 + po 1 + rs 1 = 8
        ps2_pool = top.enter_context(tc.tile_pool(name="ps2", bufs=2, space="PSUM"))
        pp = top.enter_context(tc.tile_pool(name="pp", bufs=2, space="PSUM"))
        ppo = top.enter_context(tc.tile_pool(name="ppo", bufs=1, space="PSUM"))
        ppr = top.enter_context(tc.tile_pool(name="ppr", bufs=1, space="PSUM"))

        l_tile = consts.tile([128, 128], FP32)
        nc.sync.dma_start(out=l_tile, in_=maskl)
        ones_t = consts.tile([128, 1], BF16)
        nc.sync.dma_start(out=ones_t, in_=ones)

        XG = {}

        def xdma(g):
            XG[g] = []
            for k in range(NK):
                xt = x_pool.tile([128, 512], BF16, tag="x")
                nc.scalar.dma_start(
                    out=xt,
                    in_=xT[k * 128 : (k + 1) * 128, g * 512 : (g + 1) * 512],
                )
                XG[g].append(xt)

        # All DMAs drain through the same 8 HW queues roughly in issue
        # order, so issue in first-use order: wq+x(0) interleaved (the
        # first Q chain is DMA-paced), cos/sin, then wk, wv, x(1), wp.
        WQ = []
        XG[0] = []
        cs_t = None
        for k in range(NK):
            w = wq_pool.tile([128, TPC * HD], BF16, tag="wq")
            nc.sync.dma_start(out=w, in_=wq[k * 128 : (k + 1) * 128, :])
            WQ.append(w)
            xt = x_pool.tile([128, 512], BF16, tag="x")
            nc.scalar.dma_start(
                out=xt, in_=xT[k * 128 : (k + 1) * 128, 0:512]
            )
            XG[0].append(xt)
            if k == 5:
                cs_t = consts.tile([128, T], FP32)   # [cos ; sin] halves
                nc.scalar.dma_start(out=cs_t, in_=cs)
        WK = []
        for k in range(NK):
            w = wk_pool.tile([128, TPC * HD], BF16, tag="wk")
            nc.sync.dma_start(out=w, in_=wk[k * 128 : (k + 1) * 128, :])
            WK.append(w)
        WV = []
        for k in range(NK):
            w = wv_pool.tile([128, TPC * HD], BF16, tag="wv")
            nc.sync.dma_start(out=w, in_=wv[k * 128 : (k + 1) * 128, :])
            WV.append(w)
        xdma(1)
        WP = []  # index hh*4 + m4 -> wp[hh*128:(hh+1)*128, m4*512:(m4+1)*512]
        for hh in range(TPC):
            for m4 in range(4):
                w = wp_pool.tile([128, 512], BF16, tag="wp")
                nc.sync.dma_start(
                    out=w,
                    in_=wp[hh * 128 : (hh + 1) * 128, m4 * 512 : (m4 + 1) * 512],
                )
                WP.append(w)

        QT = [qt_pool.tile([128, T], BF16, tag="qt", name=f"QT{i}") for i in range(TPC)]
        KT = [kt_pool.tile([128, T], BF16, tag="kt", name=f"KT{i}") for i in range(TPC)]
        V = [v_pool.tile([128, TPC * HD], BF16, tag="v", name=f"V{i}") for i in range(NT)]
        OT = [o_pool.tile([128, T], BF16, tag="o", name=f"OT{i}") for i in range(TPC)]

        # ---------------- unit bodies ----------------

        def qk_chain(g, hh, Wsrc, dstT):
            c0, c1 = g * 512, (g + 1) * 512
            ps = pp.tile([128, 512], FP32, tag="pp")
            for k in range(NK):
                nc.tensor.matmul(
                    ps,
                    Wsrc[k][:, hh * 128 : (hh + 1) * 128],
                    XG[g][k],
                    start=(k == 0),
                    stop=(k == NK - 1),
                )
            # rope: out_lo = q1*c - q2*s ; out_hi = q1*s + q2*c (terms
            # materialized at base 0: SB+SB operands must share a base).
            q1 = ps[0:HALF, :]
            q2 = ps[HALF:128, :]
            cT = cs_t[0:HALF, c0:c1]
            sT = cs_t[HALF:128, c0:c1]
            t1 = rope_pool.tile([HALF, 512], FP32, tag="t1")
            t2 = rope_pool.tile([HALF, 512], FP32, tag="t2")
            t3 = rope_pool.tile([HALF, 512], FP32, tag="t3")
            t4 = rope_pool.tile([HALF, 512], FP32, tag="t4")
            nc.vector.tensor_mul(t1, q1, cT)
            nc.vector.tensor_mul(t2, q2, sT)
            nc.vector.tensor_mul(t3, q1, sT)
            nc.vector.tensor_mul(t4, q2, cT)
            nc.vector.tensor_sub(dstT[hh][0:HALF, c0:c1], t1, t2)
            nc.vector.tensor_add(dstT[hh][HALF:128, c0:c1], t3, t4)

        def v_chain(g, tt):
            ps = pp.tile([128, 512], FP32, tag="pp")
            for k in range(NK):
                nc.tensor.matmul(
                    ps,
                    XG[g][k][:, tt * 128 : (tt + 1) * 128],
                    WV[k],
                    start=(k == 0),
                    stop=(k == NK - 1),
                )
            nc.scalar.copy(out=V[4 * g + tt], in_=ps)

        def a2_unit(hh, g, jp, PT2):
            """Two score chunks into one 2-bank PSUM tile + one exp."""
            c0, c1 = g * 512, (g + 1) * 512
            ps2 = ps2_pool.tile([128, 1024], FP32, tag="s2")
            pt2 = p_pool.tile([128, 1024], BF16, tag="p")
            ws = []
            for half in range(2):
                kj = 2 * jp + half
                s0 = max(0, kj - 4 * g)
                off = s0 * 128
                w = 512 - off
                base = half * 512
                nc.tensor.matmul(
                    ps2[:, base : base + w],
                    KT[hh][:, kj * 128 : (kj + 1) * 128],
                    QT[hh][:, c0 + off : c1],
                    start=True,
                    stop=True,
                )
                sd = kj - 4 * g
                if 0 <= sd <= 3:
                    dcol = sd * 128 - off
                    nc.vector.tensor_sub(
                        ps2[:, base + dcol : base + dcol + 128],
                        ps2[:, base + dcol : base + dcol + 128],
                        l_tile,
                    )
                ws.append((off, w))
            w1 = ws[1][1]
            nc.scalar.activation(out=pt2[:, : 512 + w1], in_=ps2[:, : 512 + w1],
                                 func=EXP, scale=SCALE)
            PT2.append((pt2, ws))

        def b_unit(st, kj):
            hh, g, nch, PT2 = st["hh"], st["g"], st["nch"], st["PT2"]
            if st["po"] is None:
                st["po"] = ppo.tile([128, 512], FP32, tag="po",
                                    name=f"po{g}_{hh}")
                st["rs"] = ppr.tile([1, 512], FP32, tag="rs",
                                    name=f"rsum{g}_{hh}")
            po, rs = st["po"], st["rs"]
            pt2, ws = PT2[kj // 2]
            off, w = ws[kj % 2]
            base = (kj % 2) * 512
            nc.tensor.matmul(rs[:, off:512], ones_t, pt2[:, base : base + w],
                             start=(kj == 0), stop=(kj == nch - 1))
            nc.tensor.matmul(po[:, off:512],
                             V[kj][:, hh * HD : (hh + 1) * HD],
                             pt2[:, base : base + w],
                             start=(kj == 0), stop=(kj == nch - 1))

        def b_finish(st):
            hh, g = st["hh"], st["g"]
            c0, c1 = g * 512, (g + 1) * 512
            rr = rs_pool.tile([1, 512], FP32, tag="rr")
            nc.vector.tensor_copy(out=rr, in_=st["rs"])
            ri = rs_pool.tile([1, 512], FP32, tag="ri")
            nc.vector.reciprocal_approx_fast(out=ri, in_=rr)
            rrep = rb_pool.tile([128, 512], FP32, tag="rb")
            nc.gpsimd.partition_broadcast(rrep, ri)
            nc.vector.tensor_mul(OT[hh][:, c0:c1], st["po"], rrep)

        def out_unit(g, m):
            c0, c1 = g * 512, (g + 1) * 512
            ps = pp.tile([128, 512], FP32, tag="pp")
            for hh in range(TPC):
                nc.tensor.matmul(
                    ps,
                    WP[hh * 4 + m // 4][:, (m % 4) * 128 : (m % 4 + 1) * 128],
                    OT[hh][:, c0:c1],
                    start=(hh == 0),
                    stop=(hh == TPC - 1),
                )
            ob = ob_pool.tile([128, 512], BF16, tag="ob")
            nc.scalar.copy(out=ob, in_=ps)
            nc.sync.dma_start(out=outT[m * 128 : (m + 1) * 128, c0:c1], in_=ob)

        # ---------------- unit lists ----------------

        def proj_units(g):
            us = []
            for Wsrc, dstT in ((WQ, QT), (WK, KT)):
                for hh in range(TPC):
                    us.append(lambda g=g, hh=hh, Wsrc=Wsrc, dstT=dstT:
                              qk_chain(g, hh, Wsrc, dstT))
            for tt in range(4):
                us.append(lambda g=g, tt=tt: v_chain(g, tt))
            return us

        def attn_spine(g):
            nch = 4 * g + 4
            npair = nch // 2
            units = []
            prev = None
            for hh in range(TPC):
                st = {"hh": hh, "g": g, "nch": nch, "PT2": [], "po": None,
                      "rs": None}
                for jp in range(npair):
                    units.append(lambda hh=hh, g=g, jp=jp, PT2=st["PT2"]:
                                 a2_unit(hh, g, jp, PT2))
                    if prev is not None:
                        def b2(prev=prev, jp=jp):
                            b_unit(prev, 2 * jp)
                            b_unit(prev, 2 * jp + 1)
                        units.append(b2)
                if prev is not None:
                    units.append(lambda prev=prev: b_finish(prev))
                prev = st
            for jp in range(npair):
                def b2(prev=prev, jp=jp):
                    b_unit(prev, 2 * jp)
                    b_unit(prev, 2 * jp + 1)
                units.append(b2)
            units.append(lambda prev=prev: b_finish(prev))
            return units

        def out_units(g):
            return [lambda g=g, m=m: out_unit(g, m) for m in range(NK)]

        # ---------------- schedule ----------------

        for u in proj_units(0):
            u()
        for g in range(NG):
            spine = attn_spine(g)
            extras = []
            if g >= 1:
                extras += out_units(g - 1)   # ready immediately: cover for
            if g + 1 < NG:                   # x(g+1) DMA before proj chains
                extras += proj_units(g + 1)
            for u in _merge(spine, extras):
                u()
            if g + 2 < NG:
                xdma(g + 2)
        for u in out_units(NG - 1):
            u()


_PROGRAM = None


def _get_program():
    global _PROGRAM
    if _PROGRAM is None:
        _PROGRAM = build_program()
    return _PROGRAM


def _make_in_maps(x, cos, sin, Wqkv, Wproj):
    maskl = (np.tril(np.ones((128, 128), np.float32), -1) * 1e30).astype(np.float32)
    ones = np.ones((128, 1), dtype=BF16_NP)
    cosT = np.asarray(cos, np.float32).T   # (64, T)
    sinT = np.asarray(sin, np.float32).T
    cs = np.ascontiguousarray(np.concatenate([cosT, sinT], axis=0))
    in_maps = []
    for c in range(8):
        b, hg = c // 4, c % 4
        h0 = hg * TPC
        in_maps.append({
            "xT": np.ascontiguousarray(x[b].T.astype(BF16_NP)),
            "wq": np.ascontiguousarray(
                Wqkv[:, h0 * HD : (h0 + TPC) * HD].astype(BF16_NP)),
            "wk": np.ascontiguousarray(
                Wqkv[:, D + h0 * HD : D + (h0 + TPC) * HD].astype(BF16_NP)),
            "wv": np.ascontiguousarray(
                Wqkv[:, 2 * D + h0 * HD : 2 * D + (h0 + TPC) * HD].astype(BF16_NP)),
            "wp": np.ascontiguousarray(
                Wproj[h0 * HD : (h0 + TPC) * HD, :].astype(BF16_NP)),
            "cs": cs,
            "maskl": maskl,
            "ones": ones,
        })
    return in_maps


def _combine(results):
    outs = []
    for b in range(2):
        acc = results[4 * b]["outT"].astype(np.float32)
        for hg in range(1, 4):
            acc = acc + results[4 * b + hg]["outT"].astype(np.float32)
        outs.append(acc.T)
    return np.ascontiguousarray(np.stack(outs))


def kernel(x, cos, sin, Wqkv, Wproj):
    nc = _get_program()
    in_maps = _make_in_maps(np.asarray(x, np.float32), cos, sin,
                            np.asarray(Wqkv, np.float32), np.asarray(Wproj, np.float32))
    res = run_bass_kernel_spmd(nc, in_maps, list(range(8)))
    return _combine(res.results)


def _install_ntff_shim():
    """Provide the antenv.axon_hooks registry this container lacks, wired to
    the ctypes NTFF hook from trn_agent_boot, so trace=True works."""
    import types

    if "antenv.axon_hooks" in sys.modules:
        return
    hook = None
    try:
        from trn_agent_boot.trn_boot import _ntff_profile_via_ctypes
        hook = _ntff_profile_via_ctypes("/opt/axon/libaxon_pjrt.so")
    except Exception as e:
        print("ntff shim unavailable:", e)
    mod = types.ModuleType("antenv.axon_hooks")
    mod._hook = hook
    mod.get_axon_ntff_profile_hook = lambda: mod._hook
    mod.set_axon_ntff_profile_hook = lambda h: setattr(mod, "_hook", h)
    sys.modules["antenv.axon_hooks"] = mod
    # keep artifacts local; the bucket upload path isn't available here
    import concourse.bass_utils as bu
    bu.upload_artifacts = lambda tmpdir: tmpdir


def kernel_profiled(x, cos, sin, Wqkv, Wproj, trace_cores=None, tmpdir=None):
    nc = _get_program()
    _install_ntff_shim()
    in_maps = _make_in_maps(np.asarray(x, np.float32), cos, sin,
                            np.asarray(Wqkv, np.float32), np.asarray(Wproj, np.float32))
    res = run_bass_kernel_spmd(nc, in_maps, list(range(8)), trace=True,
                               trace_cores=trace_cores, tmpdir=tmpdir)
    return _combine(res.results), res
